# revision 11
# baseline (speedup 1.0000x reference)
"""Multi-head attention (B=4, N=2048, D=1024, H=16) on 8 Trainium2 NeuronCores.

Sharding: core = (batch b = core//2, head-group g = core%2 of 8 heads).
Each core computes qkv + attention for its 8 heads and a *partial* output
projection over its 512 features; the host sums the two partials per batch
and adds the bias (the tensor-parallel unshard).

v2: all matmul operands are bf16 (inputs cast on host), and the score
matmuls exploit PE sub-array tiling: per (head-pair p, m-tile g) TWO
row-tiled matmuls run CONCURRENTLY on disjoint halves of the 128x128 array
(head 2p on rows 0-63, head 2p+1 on rows 64-127; K=64 each, no zero-padded
q). One [128,1024] PSUM tile holds S^T for both heads (512 n-cols each), so
exp shape/count is unchanged but the PE spends half the cycles on scores.

attn@v keeps the ones-column-in-v trick (out rows 65) for softmax
denominators; h0/h1 accumulate in parallel po banks (ps_o bufs=2).
Epilogue uses reciprocal_approx_fast (~5x cheaper than DVE reciprocal).

Emission is a flat gstep stream (j chunks, p pairs, g m-tiles) paced for
the ACT engine (exp is the roofline: 33.5M elements at 1 elem/lane/cycle
@1.2GHz = 293us); kT/v windows, qT chunks and projections run as
deadline-paced background thunks so the PE prelude overlaps the exp
stream instead of preceding it.
"""
import sys

sys.path.insert(0, '/opt/trn_rl_repo')

import numpy as np
import ml_dtypes

import concourse.bass as bass  # noqa: F401  (registers engines)
import concourse.mybir as mybir
import concourse.tile as tile
from concourse import bacc
from concourse.bass_utils import run_bass_kernel_spmd

dt = mybir.dt

B = 4
N = 2048          # sequence length
D = 1024          # d_model
NH = 16           # total heads
HD = 64           # head dim
NHC = 8           # heads per core
DC = NHC * HD     # 512 features per core
SCALE = HD ** -0.5

P = 128           # partitions
KB = D // P       # 8 k-blocks
NCH = N // 512    # 4 n-chunks of 512
MT = N // P       # 16 m-tiles of 128
DB = DC // P      # 4 head-pair blocks

AV_LAG = 3
PACE = 1          # background thunks per gstep beyond deadline-forced ones


def build_program(debug=False):
    nc = bacc.Bacc("TRN2", target_bir_lowering=False, debug=False,
                   enable_asserts=False, num_devices=8)

    bf16 = dt.bfloat16
    f32 = dt.float32
    f32r = dt.float32r
    Exp = mybir.ActivationFunctionType.Exp
    MULT = mybir.AluOpType.mult

    xT = nc.dram_tensor("xT", [D, N], bf16, kind="ExternalInput")
    wqT = nc.dram_tensor("wqT", [D, DC], bf16, kind="ExternalInput")
    wkT = nc.dram_tensor("wkT", [D, DC], bf16, kind="ExternalInput")
    wvT = nc.dram_tensor("wvT", [D, DC], bf16, kind="ExternalInput")
    wpT = nc.dram_tensor("wpT", [DC, D], bf16, kind="ExternalInput")
    out = nc.dram_tensor("out", [N, D], f32, kind="ExternalOutput")
    if debug:
        dbg_kT = nc.dram_tensor("dbg_kT", [P, DB, N], bf16,
                                kind="ExternalOutput")
        dbg_qT = nc.dram_tensor("dbg_qT", [P, DB, 512], bf16,
                                kind="ExternalOutput")
        dbg_v = nc.dram_tensor("dbg_v", [P, MT, NHC, HD + 1], bf16,
                               kind="ExternalOutput")
        dbg_eS = nc.dram_tensor("dbg_eS", [P, 1024], bf16,
                                kind="ExternalOutput")
        dbg_po = nc.dram_tensor("dbg_po", [HD + 1, 1024], f32,
                                kind="ExternalOutput")
        dbg_at = nc.dram_tensor("dbg_at", [P, DB, 512], bf16,
                                kind="ExternalOutput")

    with tile.TileContext(nc) as tc:
        with tc.tile_pool(name="persist", bufs=1) as persist, \
             tc.tile_pool(name="wq", bufs=1) as wq_pool, \
             tc.tile_pool(name="qTc", bufs=2) as qT_pool, \
             tc.tile_pool(name="xw", bufs=2) as xw_pool, \
             tc.tile_pool(name="ps_S", bufs=2, space="PSUM") as ps_S, \
             tc.tile_pool(name="ps_bg", bufs=2, space="PSUM") as ps_bg, \
             tc.tile_pool(name="ps_o", bufs=2, space="PSUM") as ps_o:

            # ---- persistent SBUF tensors ----
            kT_sb = persist.tile([P, DB, N], bf16, tag="kT")
            # v with a ones column per head: [m-part, m-tile, head, 65]
            v_sb = persist.tile([P, MT, NHC, HD + 1], bf16, tag="v")
            ones_r = persist.tile([P, HD], f32r, tag="ones")

            wq_sb = wq_pool.tile([P, KB, DC], bf16, tag="wq")
            nc.vector.memset(v_sb[:], 1.0)
            nc.vector.memset(ones_r[:].bitcast(f32), 1.0)

            def load_xw(j, label, fine=False):
                xw = xw_pool.tile([P, KB, 512], bf16, tag="xw",
                                  name=f"xw_{label}")
                ap = (xT.ap()[:, j * 512:(j + 1) * 512]
                      .rearrange("(kb p) n -> p kb n", p=P))
                if fine:
                    for kb in range(KB):
                        nc.sync.dma_start(xw[:, kb, :], ap[:, kb, :])
                else:
                    nc.sync.dma_start(xw[:], ap)
                return xw

            def kT_thunks(xw, w):
                """k projection for window w as thunks (4 db x 5)."""
                box = [None]
                thunks = []
                for db in range(DB):
                    def mm_t(db, kb0):
                        if kb0 == 0:
                            box[0] = ps_bg.tile([P, 512], f32, tag="bg",
                                                name=f"pk{w}_{db}")
                        for kb in (kb0, kb0 + 1):
                            nc.tensor.matmul(
                                box[0][:],
                                lhsT=wk_box[0][:, kb, db * P:(db + 1) * P],
                                rhs=xw[:, kb, :],
                                start=(kb == 0), stop=(kb == KB - 1))
                    for kb0 in range(0, KB, 2):
                        thunks.append(lambda db=db, kb0=kb0: mm_t(db, kb0))
                    def cp_t(db=db):
                        nc.vector.tensor_copy(
                            out=kT_sb[:, db, w * 512:(w + 1) * 512],
                            in_=box[0][:])
                    thunks.append(cp_t)
                return thunks

            def v_thunks(xw, w):
                """v for the 4 m-tiles of window w as thunks."""
                box = [None]
                thunks = []
                for mc in range(4):
                    m = w * 4 + mc
                    def mm_t(mc, kb0, m=m):
                        if kb0 == 0:
                            box[0] = ps_bg.tile([P, 512], f32, tag="bg",
                                                name=f"pv{m}")
                        for kb in (kb0, kb0 + 1):
                            nc.tensor.matmul(
                                box[0][:],
                                lhsT=xw[:, kb, mc * P:(mc + 1) * P],
                                rhs=wv_box[0][:, kb, :],
                                start=(kb == 0), stop=(kb == KB - 1))
                    for kb0 in range(0, KB, 2):
                        thunks.append(lambda mc=mc, kb0=kb0: mm_t(mc, kb0))
                    def cp_t(m=m):
                        nc.vector.tensor_copy(
                            out=v_sb[:, m, :, 0:HD],
                            in_=box[0][:].rearrange("p (h d) -> p h d",
                                                    h=NHC))
                    thunks.append(cp_t)
                return thunks

            qT_tiles = [None] * NCH

            def emit_qT_thunks(jn):
                """qT(jn): [128, DB, 512] bf16, heads stacked 64/64."""
                qt = qT_pool.tile([P, DB, 512], bf16, tag="qTc",
                                  name=f"qT{jn}")
                qT_tiles[jn] = qt
                xwq = load_xw(jn, f"q{jn}")
                box = [None]
                thunks = []
                for db in range(DB):
                    def mm_t(db, kb0):
                        if kb0 == 0:
                            box[0] = ps_bg.tile([P, 512], f32, tag="bg",
                                                name=f"pq{jn}_{db}")
                        for kb in (kb0, kb0 + 1):
                            nc.tensor.matmul(
                                box[0][:],
                                lhsT=wq_sb[:, kb, db * P:(db + 1) * P],
                                rhs=xwq[:, kb, :],
                                start=(kb == 0), stop=(kb == KB - 1))
                    for kb0 in range(0, KB, 2):
                        thunks.append(lambda db=db, kb0=kb0: mm_t(db, kb0))
                    def cp_t(db=db):
                        nc.vector.tensor_copy(out=qt[:, db, :], in_=box[0][:])
                    thunks.append(cp_t)
                return thunks

            def emit_proj_thunks(j):
                """Projection of chunk j as thunks (at_j is bf16)."""
                at_j = at_tiles[j]
                thunks = []
                box = [None]
                for ns in range(4):
                    for ec in range(2):
                        def mm_t(ns, ec, cb0):
                            if cb0 == 0:
                                box[0] = ps_bg.tile([P, 512], f32, tag="bg",
                                                    name=f"pp{ns}_{ec}")
                            for cb in (cb0, cb0 + 1):
                                nc.tensor.matmul(
                                    box[0][:],
                                    lhsT=at_j[:, cb, ns * P:(ns + 1) * P],
                                    rhs=wp_box[0][:, cb,
                                                  ec * 512:(ec + 1) * 512],
                                    start=(cb == 0), stop=(cb == DB - 1))
                        for cb0 in range(0, DB, 2):
                            thunks.append(
                                lambda ns=ns, ec=ec, cb0=cb0: mm_t(ns, ec, cb0))
                        def cp_t(ns=ns, ec=ec):
                            osb = out_pool.tile([P, 512], f32, tag="osb",
                                                name=f"osb{ns}_{ec}")
                            nc.vector.tensor_copy(out=osb[:], in_=box[0][:])
                            nc.sync.dma_start(
                                out.ap()[j * 512 + ns * P:
                                         j * 512 + (ns + 1) * P,
                                         ec * 512:(ec + 1) * 512],
                                osb[:])
                        thunks.append(cp_t)
                return thunks

            wk_box = [None]
            wv_box = [None]
            wp_box = [None]

            # ---- prelude: weights + window-0 kT/v, then qT chunk 0 ----
            wkv_scope = tc.tile_pool(name="wkv", bufs=1)
            wkv_pool = wkv_scope.__enter__()
            wk_sb = wkv_pool.tile([P, KB, DC], bf16, tag="wk")
            wv_sb = wkv_pool.tile([P, KB, DC], bf16, tag="wv")
            wk_box[0] = wk_sb
            wv_box[0] = wv_sb
            wk_ap = wkT.ap().rearrange("(kb p) d -> p kb d", p=P)
            wv_ap = wvT.ap().rearrange("(kb p) d -> p kb d", p=P)
            xw0 = xw_pool.tile([P, KB, 512], bf16, tag="xw", name="xw_kv0")
            xw0_ap = (xT.ap()[:, 0:512]
                      .rearrange("(kb p) n -> p kb n", p=P))
            for kb in range(KB):
                nc.sync.dma_start(wk_sb[:, kb, :], wk_ap[:, kb, :])
                nc.sync.dma_start(xw0[:, kb, :], xw0_ap[:, kb, :])
            for kb in range(KB):
                nc.sync.dma_start(wv_sb[:, kb, :], wv_ap[:, kb, :])

            for t in kT_thunks(xw0, 0):
                t()
            nc.sync.dma_start(
                wq_sb[:], wqT.ap().rearrange("(kb p) d -> p kb d", p=P))
            for t in v_thunks(xw0, 0):
                t()
            for t in emit_qT_thunks(0):
                t()

            expS_scope = tc.tile_pool(name="expS", bufs=5)
            expS_pool = expS_scope.__enter__()
            at_scope = tc.tile_pool(name="at", bufs=2)
            at_pool = at_scope.__enter__()
            small_scope = tc.tile_pool(name="small", bufs=1)
            small_pool = small_scope.__enter__()
            out_scope = tc.tile_pool(name="outsb", bufs=2)
            out_pool = out_scope.__enter__()

            # ---- flat gstep stream: (chunk j, head-pair p, m-tile g) ----
            gsteps = [(j, p, g) for j in range(NCH) for p in range(DB)
                      for g in range(MT)]
            CHUNK = DB * MT
            at_tiles = [None] * NCH
            eS_q = {}
            po_pairs = {}
            pending_p1 = []
            pending_ep = []
            # background stream: (avail_idx, deadline_idx, thunk)
            bg_stream = []
            bg_run = []

            def bg_schedule(avail, deadline, thunks):
                for t in thunks:
                    bg_stream.append([avail, deadline, t])

            def bg_tick(idx, next_idx):
                # move available thunks into the run queue
                while bg_stream and bg_stream[0][0] <= idx:
                    e = bg_stream.pop(0)
                    bg_run.append(e)
                # forced: everything whose deadline is next_idx or earlier
                while bg_run and bg_run[0][1] <= next_idx:
                    bg_run.pop(0)[2]()
                # paced
                for _ in range(PACE):
                    if bg_run:
                        bg_run.pop(0)[2]()

            def bg_drain_all(idx):
                while bg_stream and bg_stream[0][0] <= idx:
                    bg_run.append(bg_stream.pop(0))
                while bg_run:
                    bg_run.pop(0)[2]()

            def emit_S(j, p, g):
                S = ps_S.tile([P, 1024], f32, tag="S", name=f"S{p}_{g}")
                qt = qT_tiles[j]
                nc.tensor.matmul(S[:, 0:512],
                                 lhsT=kT_sb[0:HD, p, g * P:(g + 1) * P],
                                 rhs=qt[0:HD, p, :],
                                 start=True, stop=True)
                nc.tensor.matmul(S[:, 512:1024],
                                 lhsT=kT_sb[HD:P, p, g * P:(g + 1) * P],
                                 rhs=qt[HD:P, p, :],
                                 start=True, stop=True)
                return S

            def emit_av(idx2):
                j, p, g = gsteps[idx2]
                eSp = eS_q.pop((j, p, g))
                key = (j, p)
                if key not in po_pairs:
                    po_pairs[key] = [
                        ps_o.tile([P, 512], f32, tag="o",
                                  name=f"po{j}_{p}_{h}")
                        for h in range(2)]
                po0, po1 = po_pairs[key]
                nc.tensor.matmul(po0[0:HD + 1, :],
                                 lhsT=v_sb[:, g, 2 * p, :],
                                 rhs=eSp[:, 0:512],
                                 start=(g == 0), stop=(g == MT - 1))
                nc.tensor.matmul(po1[0:HD + 1, :],
                                 lhsT=v_sb[:, g, 2 * p + 1, :],
                                 rhs=eSp[:, 512:1024],
                                 start=(g == 0), stop=(g == MT - 1))
                if g == MT - 1:
                    pending_p1.append([3, po0, po1, j, p])

            def emit_epilogue_p1(po0, po1, j, p):
                oT0 = small_pool.tile([HD + 1, 512], f32, tag="oT0",
                                      name="oT0")
                oT1 = small_pool.tile([HD + 1, 512], f32, tag="oT1",
                                      name="oT1")
                nc.vector.tensor_copy(out=oT0[:], in_=po0[0:HD + 1, :])
                nc.vector.tensor_copy(out=oT1[:], in_=po1[0:HD + 1, :])
                if debug and (j, p) == (0, 0):
                    nc.sync.dma_start(dbg_po.ap()[:, 0:512], oT0[:])
                    nc.sync.dma_start(dbg_po.ap()[:, 512:1024], oT1[:])
                return oT0, oT1

            def emit_epilogue_p2(j, p, oT0, oT1):
                at = at_tiles[j]
                rcp = small_pool.tile([33, 512], f32r, tag="rcp", name="rcp")
                with nc.allow_low_precision(reason="softmax normalize bf16"):
                    nc.vector.reciprocal(rcp[0:1, :], oT0[HD:HD + 1, :])
                    nc.vector.reciprocal(rcp[32:33, :], oT1[HD:HD + 1, :])
                    bcp0 = ps_bg.tile([P, 512], f32, tag="bg", name="bcp0")
                    nc.tensor.matmul(bcp0[0:HD, :],
                                     lhsT=ones_r[0:1, :],
                                     rhs=rcp[0:1, :],
                                     start=True, stop=True)
                    nc.vector.tensor_tensor(
                        out=at[0:HD, p, :], in0=oT0[0:HD, :],
                        in1=bcp0[0:HD, :], op=MULT)
                    bcp1 = ps_bg.tile([P, 512], f32, tag="bg", name="bcp1")
                    nc.tensor.matmul(bcp1[0:HD, :],
                                     lhsT=ones_r[32:33, :],
                                     rhs=rcp[32:33, :],
                                     start=True, stop=True)
                    tmp1 = small_pool.tile([HD, 512], bf16, tag="tmp1",
                                           name="tmp1")
                    nc.vector.tensor_tensor(
                        out=tmp1[:], in0=oT1[0:HD, :],
                        in1=bcp1[0:HD, :], op=MULT)
                    nc.sync.dma_start(at[HD:P, p, :], tmp1[:])

            def drain_queues():
                for ep in pending_p1:
                    ep[0] -= 1
                while pending_p1 and pending_p1[0][0] <= 0:
                    _, po0, po1, j, p = pending_p1.pop(0)
                    oT0, oT1 = emit_epilogue_p1(po0, po1, j, p)
                    pending_ep.append([4, j, p, oT0, oT1])
                for ep in pending_ep:
                    ep[0] -= 1
                while pending_ep and pending_ep[0][0] <= 0:
                    _, j, p, oT0, oT1 = pending_ep.pop(0)
                    emit_epilogue_p2(j, p, oT0, oT1)

            def start_chunk(c, idx):
                if c == 0:
                    wp_scope = tc.tile_pool(name="wp", bufs=1)
                    wp_pool = wp_scope.__enter__()
                    wp_box.append(wp_scope)  # keep scope alive
                    wp_sb = wp_pool.tile([P, DB, D], bf16, tag="wp")
                    nc.sync.dma_start(
                        wp_sb[:],
                        wpT.ap().rearrange("(cb p) e -> p cb e", p=P))
                    wp_box[0] = wp_sb
                    # kT/v windows 1-3 stream in during chunk 0 p0; each
                    # window w is force-drained before gstep g=4w needs it
                    for w in range(1, NCH):
                        xw = load_xw(w, f"kv{w}")
                        bg_schedule(0, 4 * w, kT_thunks(xw, w))
                        bg_schedule(0, 4 * w + 2, v_thunks(xw, w))
                if debug and c == 1:
                    nc.sync.dma_start(dbg_kT.ap(), kT_sb[:])
                    nc.sync.dma_start(dbg_v.ap(), v_sb[:])
                    nc.sync.dma_start(dbg_qT.ap(), qT_tiles[0][:])
                if debug and c == 2:
                    nc.sync.dma_start(dbg_at.ap(), at_tiles[0][:])
                at_tiles[c] = at_pool.tile([P, DB, 512], bf16, tag="at",
                                           name=f"at{c}")
                if c + 1 < NCH:
                    bg_schedule(idx, idx + CHUNK, emit_qT_thunks(c + 1))
                if c >= 1:
                    bg_schedule(idx + 16, idx + CHUNK,
                                emit_proj_thunks(c - 1))

            start_chunk(0, 0)
            S_next = emit_S(*gsteps[0])
            for idx, (j, p, g) in enumerate(gsteps):
                S_cur = S_next
                eS = expS_pool.tile([P, 1024], dt.bfloat16, tag="e",
                                    name=f"eS{p}_{g}")
                nc.scalar.activation(eS[:], S_cur[:], Exp, scale=SCALE)
                if debug and idx == 0:
                    nc.sync.dma_start(dbg_eS.ap(), eS[:])
                eS_q[(j, p, g)] = eS
                if idx + 1 < len(gsteps):
                    if (idx + 1) % CHUNK == 0:
                        bg_drain_all(idx)
                        start_chunk((idx + 1) // CHUNK, idx + 1)
                    bg_tick(idx, idx + 1)
                    S_next = emit_S(*gsteps[idx + 1])
                if idx >= AV_LAG:
                    emit_av(idx - AV_LAG)
                drain_queues()

            for idx2 in range(len(gsteps) - AV_LAG, len(gsteps)):
                emit_av(idx2)
            for _, po0, po1, j, p in pending_p1:
                oT0, oT1 = emit_epilogue_p1(po0, po1, j, p)
                pending_ep.append([0, j, p, oT0, oT1])
            for _, j, p, oT0, oT1 in pending_ep:
                emit_epilogue_p2(j, p, oT0, oT1)
            bg_drain_all(10 ** 9)

            # final chunk's projection
            for t in emit_proj_thunks(NCH - 1):
                t()

            if len(wp_box) > 1:
                wp_box[1].__exit__(None, None, None)
            out_scope.__exit__(None, None, None)
            small_scope.__exit__(None, None, None)
            at_scope.__exit__(None, None, None)
            expS_scope.__exit__(None, None, None)
            wkv_scope.__exit__(None, None, None)

    nc.compile()
    return nc


_CACHE: dict = {}


def _get_program():
    if "nc" not in _CACHE:
        _CACHE["nc"] = build_program()
    return _CACHE["nc"]


def make_in_maps(x, w_qkv, w_proj):
    """Host-side sharding: per-core input dict (bf16)."""
    bf = ml_dtypes.bfloat16
    x = np.asarray(x, dtype=np.float32)
    w_qkv = np.asarray(w_qkv, dtype=np.float32)
    w_proj = np.asarray(w_proj, dtype=np.float32)
    in_maps = []
    for core in range(8):
        b, g = divmod(core, 2)
        gsl = slice(g * DC, (g + 1) * DC)
        in_maps.append({
            "xT": np.ascontiguousarray(x[b].T.astype(bf)),            # [D, N]
            "wqT": np.ascontiguousarray(w_qkv[0 * D:1 * D][gsl].T.astype(bf)),
            "wkT": np.ascontiguousarray(w_qkv[1 * D:2 * D][gsl].T.astype(bf)),
            "wvT": np.ascontiguousarray(w_qkv[2 * D:3 * D][gsl].T.astype(bf)),
            "wpT": np.ascontiguousarray(w_proj[:, gsl].T.astype(bf)),
        })
    return in_maps


def run(x, w_qkv, w_proj, b_proj, **spmd_kwargs):
    nc = _get_program()
    in_maps = make_in_maps(x, w_qkv, w_proj)
    res = run_bass_kernel_spmd(nc, in_maps, list(range(8)), **spmd_kwargs)
    b_proj = np.asarray(b_proj, dtype=np.float32)
    outp = np.empty((B, N, D), dtype=np.float32)
    for b in range(B):
        outp[b] = (res.results[2 * b]["out"] + res.results[2 * b + 1]["out"]
                   + b_proj[None, :])
    return outp, res


def kernel(x, w_qkv, w_proj, b_proj):
    outp, _ = run(x, w_qkv, w_proj, b_proj)
    return outp


# revision 15
# speedup vs baseline: 1.0784x; 1.0784x over previous
"""Multi-head attention (B=4, N=2048, D=1024, H=16) on 8 Trainium2 NeuronCores.

Sharding: core = (batch b = core//2, head-group g = core%2 of 8 heads).
Each core computes qkv + attention for its 8 heads and a *partial* output
projection over its 512 features; the host sums the two partials per batch
and adds the bias (the tensor-parallel unshard).

v2: all matmul operands are bf16 (inputs cast on host), and the score
matmuls exploit PE sub-array tiling: per (head-pair p, m-tile g) TWO
row-tiled matmuls run CONCURRENTLY on disjoint halves of the 128x128 array
(head 2p on rows 0-63, head 2p+1 on rows 64-127; K=64 each, no zero-padded
q). One [128,1024] PSUM tile holds S^T for both heads (512 n-cols each), so
exp shape/count is unchanged but the PE spends half the cycles on scores.

attn@v keeps the ones-column-in-v trick (out rows 65) for softmax
denominators; h0/h1 accumulate in parallel po banks (ps_o bufs=2).
Epilogue uses reciprocal_approx_fast (~5x cheaper than DVE reciprocal).

Emission is a flat gstep stream (j chunks, p pairs, g m-tiles) paced for
the ACT engine (exp is the roofline: 33.5M elements at 1 elem/lane/cycle
@1.2GHz = 293us); kT/v windows, qT chunks and projections run as
deadline-paced background thunks so the PE prelude overlaps the exp
stream instead of preceding it.
"""
import sys

sys.path.insert(0, '/opt/trn_rl_repo')

import numpy as np
import ml_dtypes

import concourse.bass as bass  # noqa: F401  (registers engines)
import concourse.mybir as mybir
import concourse.tile as tile
from concourse import bacc
from concourse.bass_utils import run_bass_kernel_spmd

dt = mybir.dt

B = 4
N = 2048          # sequence length
D = 1024          # d_model
NH = 16           # total heads
HD = 64           # head dim
NHC = 8           # heads per core
DC = NHC * HD     # 512 features per core
SCALE = HD ** -0.5

P = 128           # partitions
KB = D // P       # 8 k-blocks
NCH = N // 512    # 4 n-chunks of 512
MT = N // P       # 16 m-tiles of 128
DB = DC // P      # 4 head-pair blocks

AV_LAG = 3
PACE = 1          # background thunks per gstep beyond deadline-forced ones


def build_program(debug=False):
    nc = bacc.Bacc("TRN2", target_bir_lowering=False, debug=False,
                   enable_asserts=False, num_devices=8)

    bf16 = dt.bfloat16
    f32 = dt.float32
    f32r = dt.float32r
    Exp = mybir.ActivationFunctionType.Exp
    MULT = mybir.AluOpType.mult

    xT = nc.dram_tensor("xT", [D, N], bf16, kind="ExternalInput")
    wqT = nc.dram_tensor("wqT", [D, DC], bf16, kind="ExternalInput")
    wkT = nc.dram_tensor("wkT", [D, DC], bf16, kind="ExternalInput")
    wvT = nc.dram_tensor("wvT", [D, DC], bf16, kind="ExternalInput")
    wpT = nc.dram_tensor("wpT", [DC, D], bf16, kind="ExternalInput")
    out = nc.dram_tensor("out", [N, D], f32, kind="ExternalOutput")
    if debug:
        dbg_kT = nc.dram_tensor("dbg_kT", [P, DB, N], bf16,
                                kind="ExternalOutput")
        dbg_qT = nc.dram_tensor("dbg_qT", [P, DB, 512], bf16,
                                kind="ExternalOutput")
        dbg_v = nc.dram_tensor("dbg_v", [P, MT, NHC, HD + 1], bf16,
                               kind="ExternalOutput")
        dbg_eS = nc.dram_tensor("dbg_eS", [P, 1024], bf16,
                                kind="ExternalOutput")
        dbg_po = nc.dram_tensor("dbg_po", [HD + 1, 1024], f32,
                                kind="ExternalOutput")
        dbg_at = nc.dram_tensor("dbg_at", [P, DB, 512], bf16,
                                kind="ExternalOutput")

    with tile.TileContext(nc) as tc:
        with tc.tile_pool(name="persist", bufs=1) as persist, \
             tc.tile_pool(name="wq", bufs=1) as wq_pool, \
             tc.tile_pool(name="qTc", bufs=2) as qT_pool, \
             tc.tile_pool(name="xw", bufs=2) as xw_pool, \
             tc.tile_pool(name="ps_S", bufs=2, space="PSUM") as ps_S, \
             tc.tile_pool(name="ps_bg", bufs=2, space="PSUM") as ps_bg, \
             tc.tile_pool(name="ps_o", bufs=2, space="PSUM") as ps_o:

            # ---- persistent SBUF tensors ----
            kT_sb = persist.tile([P, DB, N], bf16, tag="kT")
            # v with a ones column per head: [m-part, m-tile, head, 65]
            v_sb = persist.tile([P, MT, NHC, HD + 1], bf16, tag="v")
            ones_r = persist.tile([P, HD], f32r, tag="ones")

            wq_sb = wq_pool.tile([P, KB, DC], bf16, tag="wq")
            nc.vector.memset(v_sb[:], 1.0)
            nc.vector.memset(ones_r[:].bitcast(f32), 1.0)

            def load_xw(j, label, fine=False):
                xw = xw_pool.tile([P, KB, 512], bf16, tag="xw",
                                  name=f"xw_{label}")
                ap = (xT.ap()[:, j * 512:(j + 1) * 512]
                      .rearrange("(kb p) n -> p kb n", p=P))
                if fine:
                    for kb in range(KB):
                        nc.sync.dma_start(xw[:, kb, :], ap[:, kb, :])
                else:
                    nc.sync.dma_start(xw[:], ap)
                return xw

            def kT_thunks(xw, w):
                """k projection for window w as thunks (4 db x 5)."""
                box = [None]
                thunks = []
                for db in range(DB):
                    def mm_t(db, kb0):
                        if kb0 == 0:
                            box[0] = ps_bg.tile([P, 512], f32, tag="bg",
                                                name=f"pk{w}_{db}")
                        for kb in (kb0, kb0 + 1):
                            nc.tensor.matmul(
                                box[0][:],
                                lhsT=wk_box[0][:, kb, db * P:(db + 1) * P],
                                rhs=xw[:, kb, :],
                                start=(kb == 0), stop=(kb == KB - 1))
                    for kb0 in range(0, KB, 2):
                        thunks.append(lambda db=db, kb0=kb0: mm_t(db, kb0))
                    def cp_t(db=db):
                        nc.vector.tensor_copy(
                            out=kT_sb[:, db, w * 512:(w + 1) * 512],
                            in_=box[0][:])
                    thunks.append(cp_t)
                return thunks

            def v_thunks(xw, w):
                """v for the 4 m-tiles of window w as thunks."""
                box = [None]
                thunks = []
                for mc in range(4):
                    m = w * 4 + mc
                    def mm_t(mc, kb0, m=m):
                        if kb0 == 0:
                            box[0] = ps_bg.tile([P, 512], f32, tag="bg",
                                                name=f"pv{m}")
                        for kb in (kb0, kb0 + 1):
                            nc.tensor.matmul(
                                box[0][:],
                                lhsT=xw[:, kb, mc * P:(mc + 1) * P],
                                rhs=wv_box[0][:, kb, :],
                                start=(kb == 0), stop=(kb == KB - 1))
                    for kb0 in range(0, KB, 2):
                        thunks.append(lambda mc=mc, kb0=kb0: mm_t(mc, kb0))
                    def cp_t(m=m):
                        nc.vector.tensor_copy(
                            out=v_sb[:, m, :, 0:HD],
                            in_=box[0][:].rearrange("p (h d) -> p h d",
                                                    h=NHC))
                    thunks.append(cp_t)
                return thunks

            qT_tiles = [None] * NCH

            def emit_qT_thunks(jn):
                """qT(jn): [128, DB, 512] bf16, heads stacked 64/64."""
                qt = qT_pool.tile([P, DB, 512], bf16, tag="qTc",
                                  name=f"qT{jn}")
                qT_tiles[jn] = qt
                xwq = load_xw(jn, f"q{jn}")
                box = [None]
                thunks = []
                for db in range(DB):
                    def mm_t(db, kb0):
                        if kb0 == 0:
                            box[0] = ps_bg.tile([P, 512], f32, tag="bg",
                                                name=f"pq{jn}_{db}")
                        for kb in (kb0, kb0 + 1):
                            nc.tensor.matmul(
                                box[0][:],
                                lhsT=wq_sb[:, kb, db * P:(db + 1) * P],
                                rhs=xwq[:, kb, :],
                                start=(kb == 0), stop=(kb == KB - 1))
                    for kb0 in range(0, KB, 2):
                        thunks.append(lambda db=db, kb0=kb0: mm_t(db, kb0))
                    def cp_t(db=db):
                        nc.vector.tensor_copy(out=qt[:, db, :], in_=box[0][:])
                    thunks.append(cp_t)
                return thunks

            def emit_proj_thunks(j):
                """Projection of chunk j as thunks (at_j is bf16)."""
                at_j = at_tiles[j]
                thunks = []
                box = [None]
                for ns in range(4):
                    for ec in range(2):
                        def mm_t(ns, ec, cb0):
                            if cb0 == 0:
                                box[0] = ps_bg.tile([P, 512], f32, tag="bg",
                                                    name=f"pp{ns}_{ec}")
                            for cb in (cb0, cb0 + 1):
                                nc.tensor.matmul(
                                    box[0][:],
                                    lhsT=at_j[:, cb, ns * P:(ns + 1) * P],
                                    rhs=wp_box[0][:, cb,
                                                  ec * 512:(ec + 1) * 512],
                                    start=(cb == 0), stop=(cb == DB - 1))
                        for cb0 in range(0, DB, 2):
                            thunks.append(
                                lambda ns=ns, ec=ec, cb0=cb0: mm_t(ns, ec, cb0))
                        def cp_t(ns=ns, ec=ec):
                            osb = out_pool.tile([P, 512], f32, tag="osb",
                                                name=f"osb{ns}_{ec}")
                            nc.vector.tensor_copy(out=osb[:], in_=box[0][:])
                            nc.sync.dma_start(
                                out.ap()[j * 512 + ns * P:
                                         j * 512 + (ns + 1) * P,
                                         ec * 512:(ec + 1) * 512],
                                osb[:])
                        thunks.append(cp_t)
                return thunks

            wk_box = [None]
            wv_box = [None]
            wp_box = [None]

            # ---- prelude: weights + window-0 kT/v, then qT chunk 0 ----
            wkv_scope = tc.tile_pool(name="wkv", bufs=1)
            wkv_pool = wkv_scope.__enter__()
            wk_sb = wkv_pool.tile([P, KB, DC], bf16, tag="wk")
            wv_sb = wkv_pool.tile([P, KB, DC], bf16, tag="wv")
            wk_box[0] = wk_sb
            wv_box[0] = wv_sb
            wk_ap = wkT.ap().rearrange("(kb p) d -> p kb d", p=P)
            wv_ap = wvT.ap().rearrange("(kb p) d -> p kb d", p=P)
            xw0 = xw_pool.tile([P, KB, 512], bf16, tag="xw", name="xw_kv0")
            xw0_ap = (xT.ap()[:, 0:512]
                      .rearrange("(kb p) n -> p kb n", p=P))
            for kb in range(KB):
                nc.sync.dma_start(wk_sb[:, kb, :], wk_ap[:, kb, :])
                nc.sync.dma_start(xw0[:, kb, :], xw0_ap[:, kb, :])
            for kb in range(KB):
                nc.sync.dma_start(wv_sb[:, kb, :], wv_ap[:, kb, :])

            for t in kT_thunks(xw0, 0):
                t()
            nc.sync.dma_start(
                wq_sb[:], wqT.ap().rearrange("(kb p) d -> p kb d", p=P))
            for t in v_thunks(xw0, 0):
                t()
            for t in emit_qT_thunks(0):
                t()

            expS_scope = tc.tile_pool(name="expS", bufs=5)
            expS_pool = expS_scope.__enter__()
            at_scope = tc.tile_pool(name="at", bufs=2)
            at_pool = at_scope.__enter__()
            small_scope = tc.tile_pool(name="small", bufs=1)
            small_pool = small_scope.__enter__()
            out_scope = tc.tile_pool(name="outsb", bufs=2)
            out_pool = out_scope.__enter__()

            # ---- flat gstep stream: (chunk j, head-pair p, m-tile g) ----
            gsteps = [(j, p, g) for j in range(NCH) for p in range(DB)
                      for g in range(MT)]
            CHUNK = DB * MT
            at_tiles = [None] * NCH
            eS_q = {}
            po_pairs = {}
            pending_p1 = []
            pending_ep = []
            # background stream: (avail_idx, deadline_idx, thunk)
            bg_stream = []
            bg_run = []

            def bg_schedule(avail, deadline, thunks):
                for t in thunks:
                    bg_stream.append([avail, deadline, t])

            def bg_tick(idx, next_idx):
                # move available thunks into the run queue
                while bg_stream and bg_stream[0][0] <= idx:
                    e = bg_stream.pop(0)
                    bg_run.append(e)
                # forced: everything whose deadline is next_idx or earlier
                while bg_run and bg_run[0][1] <= next_idx:
                    bg_run.pop(0)[2]()
                # paced
                for _ in range(PACE):
                    if bg_run:
                        bg_run.pop(0)[2]()

            def bg_drain_all(idx):
                while bg_stream and bg_stream[0][0] <= idx:
                    bg_run.append(bg_stream.pop(0))
                while bg_run:
                    bg_run.pop(0)[2]()

            def emit_S(j, p, g):
                S = ps_S.tile([P, 1024], f32, tag="S", name=f"S{p}_{g}")
                qt = qT_tiles[j]
                nc.tensor.matmul(S[:, 0:512],
                                 lhsT=kT_sb[0:HD, p, g * P:(g + 1) * P],
                                 rhs=qt[0:HD, p, :],
                                 start=True, stop=True)
                nc.tensor.matmul(S[:, 512:1024],
                                 lhsT=kT_sb[HD:P, p, g * P:(g + 1) * P],
                                 rhs=qt[HD:P, p, :],
                                 start=True, stop=True)
                return S

            def emit_av(idx2):
                j, p, g = gsteps[idx2]
                eSp = eS_q.pop((j, p, g))
                key = (j, p)
                if key not in po_pairs:
                    po_pairs[key] = [
                        ps_o.tile([P, 512], f32, tag="o",
                                  name=f"po{j}_{p}_{h}")
                        for h in range(2)]
                po0, po1 = po_pairs[key]
                nc.tensor.matmul(po0[0:HD + 1, :],
                                 lhsT=v_sb[:, g, 2 * p, :],
                                 rhs=eSp[:, 0:512],
                                 start=(g == 0), stop=(g == MT - 1))
                nc.tensor.matmul(po1[0:HD + 1, :],
                                 lhsT=v_sb[:, g, 2 * p + 1, :],
                                 rhs=eSp[:, 512:1024],
                                 start=(g == 0), stop=(g == MT - 1))
                if g == MT - 1:
                    pending_p1.append([3, po0, po1, j, p])

            def emit_epilogue_p1(po0, po1, j, p):
                oT0 = small_pool.tile([HD + 1, 512], f32, tag="oT0",
                                      name="oT0")
                oT1 = small_pool.tile([HD + 1, 512], f32, tag="oT1",
                                      name="oT1")
                nc.vector.tensor_copy(out=oT0[:], in_=po0[0:HD + 1, :])
                nc.vector.tensor_copy(out=oT1[:], in_=po1[0:HD + 1, :])
                if debug and (j, p) == (0, 0):
                    nc.sync.dma_start(dbg_po.ap()[:, 0:512], oT0[:])
                    nc.sync.dma_start(dbg_po.ap()[:, 512:1024], oT1[:])
                return oT0, oT1

            def emit_epilogue_recip(oT0, oT1):
                rcp = small_pool.tile([33, 512], f32r, tag="rcp", name="rcp")
                with nc.allow_low_precision(reason="softmax recip to f32r"):
                    nc.vector.reciprocal(rcp[0:1, :], oT0[HD:HD + 1, :])
                    nc.vector.reciprocal(rcp[32:33, :], oT1[HD:HD + 1, :])
                return rcp

            def emit_epilogue_p2(j, p, oT0, oT1, rcp):
                at = at_tiles[j]
                with nc.allow_low_precision(reason="softmax normalize bf16"):
                    bcp0 = ps_bg.tile([P, 512], f32, tag="bg", name="bcp0")
                    nc.tensor.matmul(bcp0[0:HD, :],
                                     lhsT=ones_r[0:1, :],
                                     rhs=rcp[0:1, :],
                                     start=True, stop=True)
                    nc.vector.tensor_tensor(
                        out=at[0:HD, p, :], in0=oT0[0:HD, :],
                        in1=bcp0[0:HD, :], op=MULT)
                    bcp1 = ps_bg.tile([P, 512], f32, tag="bg", name="bcp1")
                    nc.tensor.matmul(bcp1[0:HD, :],
                                     lhsT=ones_r[32:33, :],
                                     rhs=rcp[32:33, :],
                                     start=True, stop=True)
                    tmp1 = small_pool.tile([HD, 512], bf16, tag="tmp1",
                                           name="tmp1")
                    nc.vector.tensor_tensor(
                        out=tmp1[:], in0=oT1[0:HD, :],
                        in1=bcp1[0:HD, :], op=MULT)
                    nc.sync.dma_start(at[HD:P, p, :], tmp1[:])

            def drain_queues():
                for ep in pending_p1:
                    ep[0] -= 1
                while pending_p1 and pending_p1[0][0] <= 0:
                    _, po0, po1, j, p = pending_p1.pop(0)
                    oT0, oT1 = emit_epilogue_p1(po0, po1, j, p)
                    rcp = emit_epilogue_recip(oT0, oT1)
                    # bcp matmuls enter the in-order PE queue only after the
                    # 2x3.3us DVE reciprocals have surely finished
                    pending_ep.append([8, j, p, oT0, oT1, rcp])
                for ep in pending_ep:
                    ep[0] -= 1
                while pending_ep and pending_ep[0][0] <= 0:
                    _, j, p, oT0, oT1, rcp = pending_ep.pop(0)
                    emit_epilogue_p2(j, p, oT0, oT1, rcp)

            def start_chunk(c, idx):
                if c == 0:
                    wp_scope = tc.tile_pool(name="wp", bufs=1)
                    wp_pool = wp_scope.__enter__()
                    wp_box.append(wp_scope)  # keep scope alive
                    wp_sb = wp_pool.tile([P, DB, D], bf16, tag="wp")
                    nc.sync.dma_start(
                        wp_sb[:],
                        wpT.ap().rearrange("(cb p) e -> p cb e", p=P))
                    wp_box[0] = wp_sb
                    # kT/v windows 1-3 stream in during chunk 0 p0; each
                    # window w is force-drained before gstep g=4w needs it
                    for w in range(1, NCH):
                        xw = load_xw(w, f"kv{w}")
                        bg_schedule(0, 4 * w, kT_thunks(xw, w))
                        bg_schedule(0, 4 * w + 2, v_thunks(xw, w))
                if debug and c == 1:
                    nc.sync.dma_start(dbg_kT.ap(), kT_sb[:])
                    nc.sync.dma_start(dbg_v.ap(), v_sb[:])
                    nc.sync.dma_start(dbg_qT.ap(), qT_tiles[0][:])
                if debug and c == 2:
                    nc.sync.dma_start(dbg_at.ap(), at_tiles[0][:])
                at_tiles[c] = at_pool.tile([P, DB, 512], bf16, tag="at",
                                           name=f"at{c}")
                if c + 1 < NCH:
                    bg_schedule(idx, idx + CHUNK, emit_qT_thunks(c + 1))
                if c >= 1:
                    bg_schedule(idx + 16, idx + CHUNK,
                                emit_proj_thunks(c - 1))

            start_chunk(0, 0)
            S_next = emit_S(*gsteps[0])
            for idx, (j, p, g) in enumerate(gsteps):
                S_cur = S_next
                eS = expS_pool.tile([P, 1024], dt.bfloat16, tag="e",
                                    name=f"eS{p}_{g}")
                nc.scalar.activation(eS[:], S_cur[:], Exp, scale=SCALE)
                if debug and idx == 0:
                    nc.sync.dma_start(dbg_eS.ap(), eS[:])
                eS_q[(j, p, g)] = eS
                if idx >= AV_LAG:
                    emit_av(idx - AV_LAG)
                drain_queues()
                if idx + 1 < len(gsteps):
                    if (idx + 1) % CHUNK == 0:
                        bg_drain_all(idx)
                        start_chunk((idx + 1) // CHUNK, idx + 1)
                    bg_tick(idx, idx + 1)
                    S_next = emit_S(*gsteps[idx + 1])

            for idx2 in range(len(gsteps) - AV_LAG, len(gsteps)):
                emit_av(idx2)
            for _, po0, po1, j, p in pending_p1:
                oT0, oT1 = emit_epilogue_p1(po0, po1, j, p)
                rcp = emit_epilogue_recip(oT0, oT1)
                pending_ep.append([0, j, p, oT0, oT1, rcp])
            for _, j, p, oT0, oT1, rcp in pending_ep:
                emit_epilogue_p2(j, p, oT0, oT1, rcp)
            bg_drain_all(10 ** 9)

            # final chunk's projection
            for t in emit_proj_thunks(NCH - 1):
                t()

            if len(wp_box) > 1:
                wp_box[1].__exit__(None, None, None)
            out_scope.__exit__(None, None, None)
            small_scope.__exit__(None, None, None)
            at_scope.__exit__(None, None, None)
            expS_scope.__exit__(None, None, None)
            wkv_scope.__exit__(None, None, None)

    nc.compile()
    return nc


_CACHE: dict = {}


def _get_program():
    if "nc" not in _CACHE:
        _CACHE["nc"] = build_program()
    return _CACHE["nc"]


def make_in_maps(x, w_qkv, w_proj):
    """Host-side sharding: per-core input dict (bf16)."""
    bf = ml_dtypes.bfloat16
    x = np.asarray(x, dtype=np.float32)
    w_qkv = np.asarray(w_qkv, dtype=np.float32)
    w_proj = np.asarray(w_proj, dtype=np.float32)
    in_maps = []
    for core in range(8):
        b, g = divmod(core, 2)
        gsl = slice(g * DC, (g + 1) * DC)
        in_maps.append({
            "xT": np.ascontiguousarray(x[b].T.astype(bf)),            # [D, N]
            "wqT": np.ascontiguousarray(w_qkv[0 * D:1 * D][gsl].T.astype(bf)),
            "wkT": np.ascontiguousarray(w_qkv[1 * D:2 * D][gsl].T.astype(bf)),
            "wvT": np.ascontiguousarray(w_qkv[2 * D:3 * D][gsl].T.astype(bf)),
            "wpT": np.ascontiguousarray(w_proj[:, gsl].T.astype(bf)),
        })
    return in_maps


def run(x, w_qkv, w_proj, b_proj, **spmd_kwargs):
    nc = _get_program()
    in_maps = make_in_maps(x, w_qkv, w_proj)
    res = run_bass_kernel_spmd(nc, in_maps, list(range(8)), **spmd_kwargs)
    b_proj = np.asarray(b_proj, dtype=np.float32)
    outp = np.empty((B, N, D), dtype=np.float32)
    for b in range(B):
        outp[b] = (res.results[2 * b]["out"] + res.results[2 * b + 1]["out"]
                   + b_proj[None, :])
    return outp, res


def kernel(x, w_qkv, w_proj, b_proj):
    outp, _ = run(x, w_qkv, w_proj, b_proj)
    return outp


# revision 18
# speedup vs baseline: 1.0920x; 1.0126x over previous
"""Multi-head attention (B=4, N=2048, D=1024, H=16) on 8 Trainium2 NeuronCores.

Sharding: core = (batch b = core//2, head-group g = core%2 of 8 heads).
Each core computes qkv + attention for its 8 heads and a *partial* output
projection over its 512 features; the host sums the two partials per batch
and adds the bias (the tensor-parallel unshard).

All matmul operands are bf16 (inputs cast on host). Score matmuls exploit
PE sub-array tiling: per (head-pair p, m-tile g) TWO row-tiled matmuls run
CONCURRENTLY on disjoint halves of the 128x128 array (head 2p contracts on
rows 0-63, head 2p+1 on rows 64-127; K=64 each, no zero-padded q). One
[128,1024] PSUM tile holds S^T for both heads (512 n-cols each), so exp
shape/count is unchanged but the PE spends half the cycles on scores.

attn@v keeps the ones-column-in-v trick (out rows 65) for softmax
denominators; h0/h1 accumulate in parallel po banks (ps_o bufs=2).

Emission is a flat gstep stream (chunk j, pair p, m-tile g) paced for the
ACT engine (exp is the roofline: 33.5M elements at 1/lane/cycle @1.2GHz =
293us). All qkv/projection work runs as EDF-scheduled background thunks:
each kT window/pair-block, v m-tile, and q block carries the gstep index
deadline at which the foreground stream first needs it, so the first exp
fires ~10us in and k/v production for windows 1-3 overlaps chunk 0's exp
stream. Pair epilogues are staged (oT copies at +3, denominator DMAs into
a [33,512] staging tile + ONE batched DVE reciprocal at +5, ones-broadcast
bcp matmuls + normalize multiplies at +10) so the 3.3us reciprocal never
head-of-line-blocks the in-order PE queue.
"""
import heapq
import sys

sys.path.insert(0, '/opt/trn_rl_repo')

import numpy as np
import ml_dtypes

import concourse.bass as bass  # noqa: F401  (registers engines)
import concourse.mybir as mybir
import concourse.tile as tile
from concourse import bacc
from concourse.bass_utils import run_bass_kernel_spmd

dt = mybir.dt

B = 4
N = 2048          # sequence length
D = 1024          # d_model
NH = 16           # total heads
HD = 64           # head dim
NHC = 8           # heads per core
DC = NHC * HD     # 512 features per core
SCALE = HD ** -0.5

P = 128           # partitions
KB = D // P       # 8 k-blocks
NCH = N // 512    # 4 n-chunks of 512
MT = N // P       # 16 m-tiles of 128
DB = DC // P      # 4 head-pair blocks

AV_LAG = 3
PACE = 1          # paced background thunks per gstep beyond deadline-forced


def build_program(debug=False):
    nc = bacc.Bacc("TRN2", target_bir_lowering=False, debug=False,
                   enable_asserts=False, num_devices=8)

    bf16 = dt.bfloat16
    f32 = dt.float32
    f32r = dt.float32r
    Exp = mybir.ActivationFunctionType.Exp
    MULT = mybir.AluOpType.mult

    xT = nc.dram_tensor("xT", [D, N], bf16, kind="ExternalInput")
    wqT = nc.dram_tensor("wqT", [D, DC], bf16, kind="ExternalInput")
    wkT = nc.dram_tensor("wkT", [D, DC], bf16, kind="ExternalInput")
    wvT = nc.dram_tensor("wvT", [D, DC], bf16, kind="ExternalInput")
    wpT = nc.dram_tensor("wpT", [DC, D], bf16, kind="ExternalInput")
    out = nc.dram_tensor("out", [N, D], f32, kind="ExternalOutput")
    if debug:
        dbg_kT = nc.dram_tensor("dbg_kT", [P, DB, N], bf16,
                                kind="ExternalOutput")
        dbg_qT = nc.dram_tensor("dbg_qT", [P, DB, 512], bf16,
                                kind="ExternalOutput")
        dbg_v = nc.dram_tensor("dbg_v", [P, MT, NHC, HD + 1], bf16,
                               kind="ExternalOutput")
        dbg_eS = nc.dram_tensor("dbg_eS", [P, 1024], bf16,
                                kind="ExternalOutput")
        dbg_po = nc.dram_tensor("dbg_po", [HD + 1, 1024], f32,
                                kind="ExternalOutput")
        dbg_at = nc.dram_tensor("dbg_at", [P, DB, 512], bf16,
                                kind="ExternalOutput")

    with tile.TileContext(nc) as tc:
        with tc.tile_pool(name="persist", bufs=1) as persist, \
             tc.tile_pool(name="wq", bufs=1) as wq_pool, \
             tc.tile_pool(name="qTc", bufs=2) as qT_pool, \
             tc.tile_pool(name="xw", bufs=6) as xw_pool, \
             tc.tile_pool(name="ps_S", bufs=2, space="PSUM") as ps_S, \
             tc.tile_pool(name="ps_bg", bufs=2, space="PSUM") as ps_bg, \
             tc.tile_pool(name="ps_o", bufs=2, space="PSUM") as ps_o:

            # ---- persistent SBUF tensors ----
            kT_sb = persist.tile([P, DB, N], bf16, tag="kT")
            # v with a ones column per head: [m-part, m-tile, head, 65]
            v_sb = persist.tile([P, MT, NHC, HD + 1], bf16, tag="v")
            ones_r = persist.tile([P, HD], f32r, tag="ones")
            # denominator staging: pair's two softmax-denominator rows land
            # on partitions 0 and 32; rows 1-31 stay 1.0 so one batched
            # reciprocal covers both heads
            den_sb = persist.tile([33, 512], f32, tag="den")

            wq_sb = wq_pool.tile([P, KB, DC], bf16, tag="wq")
            nc.vector.memset(v_sb[:], 1.0)
            nc.vector.memset(ones_r[:].bitcast(f32), 1.0)
            nc.vector.memset(den_sb[:], 1.0)

            def load_xw(j, label, fine=False):
                xw = xw_pool.tile([P, KB, 512], bf16, tag="xw",
                                  name=f"xw_{label}")
                ap = (xT.ap()[:, j * 512:(j + 1) * 512]
                      .rearrange("(kb p) n -> p kb n", p=P))
                if fine:
                    for kb in range(KB):
                        nc.sync.dma_start(xw[:, kb, :], ap[:, kb, :])
                else:
                    nc.sync.dma_start(xw[:], ap)
                return xw

            # ---- background thunk machinery (EDF) ----
            bg_wait = []          # appended in nondecreasing avail order

            bg_heap = []          # (deadline, seq, thunk)
            bg_seq = [0]

            def bg_schedule(avail, deadline, thunks):
                for t in thunks:
                    bg_wait.append((avail, deadline, bg_seq[0], t))
                    bg_seq[0] += 1

            def bg_tick(idx, next_idx, pace=PACE):
                while bg_wait and bg_wait[0][0] <= idx:
                    _, dl, seq, t = bg_wait.pop(0)
                    heapq.heappush(bg_heap, (dl, seq, t))
                while bg_heap and bg_heap[0][0] <= next_idx:
                    heapq.heappop(bg_heap)[2]()
                for _ in range(pace):
                    if bg_heap:
                        heapq.heappop(bg_heap)[2]()

            def bg_drain_all(idx):
                while bg_wait and bg_wait[0][0] <= idx:
                    _, dl, seq, t = bg_wait.pop(0)
                    heapq.heappush(bg_heap, (dl, seq, t))
                while bg_heap:
                    heapq.heappop(bg_heap)[2]()

            # ---- qkv production thunks ----
            def kT_db_thunks(xw_box, w, db):
                """k projection block (window w, pair db): 4 mm + 1 copy."""
                box = [None]
                thunks = []

                def mm_t(kb0):
                    if kb0 == 0:
                        box[0] = ps_bg.tile([P, 512], f32, tag="bg",
                                            name=f"pk{w}_{db}")
                    for kb in (kb0, kb0 + 1):
                        nc.tensor.matmul(
                            box[0][:],
                            lhsT=wk_box[0][:, kb, db * P:(db + 1) * P],
                            rhs=xw_box[0][:, kb, :],
                            start=(kb == 0), stop=(kb == KB - 1))
                for kb0 in range(0, KB, 2):
                    thunks.append(lambda kb0=kb0: mm_t(kb0))

                def cp_t():
                    nc.vector.tensor_copy(
                        out=kT_sb[:, db, w * 512:(w + 1) * 512],
                        in_=box[0][:])
                thunks.append(cp_t)
                return thunks

            def v_m_thunks(xw_box, w, mc):
                """v for m-tile 4w+mc: 4 mm + 1 copy."""
                m = w * 4 + mc
                box = [None]
                thunks = []

                def mm_t(kb0):
                    if kb0 == 0:
                        box[0] = ps_bg.tile([P, 512], f32, tag="bg",
                                            name=f"pv{m}")
                    for kb in (kb0, kb0 + 1):
                        nc.tensor.matmul(
                            box[0][:],
                            lhsT=xw_box[0][:, kb, mc * P:(mc + 1) * P],
                            rhs=wv_box[0][:, kb, :],
                            start=(kb == 0), stop=(kb == KB - 1))
                for kb0 in range(0, KB, 2):
                    thunks.append(lambda kb0=kb0: mm_t(kb0))

                def cp_t():
                    nc.vector.tensor_copy(
                        out=v_sb[:, m, :, 0:HD],
                        in_=box[0][:].rearrange("p (h d) -> p h d", h=NHC))
                thunks.append(cp_t)
                return thunks

            qT_tiles = [None] * NCH

            def q_db_thunks(jn, xw_box, qt, db):
                """q block db of chunk jn: 4 mm + 1 copy (no zero pad)."""
                box = [None]
                thunks = []

                def mm_t(kb0):
                    if kb0 == 0:
                        box[0] = ps_bg.tile([P, 512], f32, tag="bg",
                                            name=f"pq{jn}_{db}")
                    for kb in (kb0, kb0 + 1):
                        nc.tensor.matmul(
                            box[0][:],
                            lhsT=wq_sb[:, kb, db * P:(db + 1) * P],
                            rhs=xw_box[0][:, kb, :],
                            start=(kb == 0), stop=(kb == KB - 1))
                for kb0 in range(0, KB, 2):
                    thunks.append(lambda kb0=kb0: mm_t(kb0))

                def cp_t():
                    nc.vector.tensor_copy(out=qt[:, db, :], in_=box[0][:])
                thunks.append(cp_t)
                return thunks

            def schedule_qT(jn, avail, deadline):
                qt = qT_pool.tile([P, DB, 512], bf16, tag="qTc",
                                  name=f"qT{jn}")
                qT_tiles[jn] = qt
                xw_box = [None]

                def load_t():
                    xw_box[0] = load_xw(jn, f"q{jn}")
                thunks = [load_t]
                for db in range(DB):
                    thunks.extend(q_db_thunks(jn, xw_box, qt, db))
                bg_schedule(avail, deadline, thunks)

            def emit_proj_thunks(j):
                """Projection of chunk j (at_j is bf16)."""
                at_j = at_tiles[j]
                thunks = []
                box = [None]
                for ns in range(4):
                    for ec in range(2):
                        def mm_t(ns, ec, cb0):
                            if cb0 == 0:
                                box[0] = ps_bg.tile([P, 512], f32, tag="bg",
                                                    name=f"pp{ns}_{ec}")
                            for cb in (cb0, cb0 + 1):
                                nc.tensor.matmul(
                                    box[0][:],
                                    lhsT=at_j[:, cb, ns * P:(ns + 1) * P],
                                    rhs=wp_box[0][:, cb,
                                                  ec * 512:(ec + 1) * 512],
                                    start=(cb == 0), stop=(cb == DB - 1))
                        for cb0 in range(0, DB, 2):
                            thunks.append(
                                lambda ns=ns, ec=ec, cb0=cb0: mm_t(ns, ec, cb0))
                        def cp_t(ns=ns, ec=ec):
                            osb = out_pool.tile([P, 512], f32, tag="osb",
                                                name=f"osb{ns}_{ec}")
                            nc.vector.tensor_copy(out=osb[:], in_=box[0][:])
                            nc.sync.dma_start(
                                out.ap()[j * 512 + ns * P:
                                         j * 512 + (ns + 1) * P,
                                         ec * 512:(ec + 1) * 512],
                                osb[:])
                        thunks.append(cp_t)
                return thunks

            wk_box = [None]
            wv_box = [None]
            wp_box = [None]
            xw_boxes = [[None] for _ in range(NCH)]

            # ---- prelude: DMAs + minimal serial work for gstep 0 ----
            wkv_scope = tc.tile_pool(name="wkv", bufs=1)
            wkv_pool = wkv_scope.__enter__()
            wk_sb = wkv_pool.tile([P, KB, DC], bf16, tag="wk")
            wv_sb = wkv_pool.tile([P, KB, DC], bf16, tag="wv")
            wk_box[0] = wk_sb
            wv_box[0] = wv_sb
            wk_ap = wkT.ap().rearrange("(kb p) d -> p kb d", p=P)
            wv_ap = wvT.ap().rearrange("(kb p) d -> p kb d", p=P)
            xw0 = xw_pool.tile([P, KB, 512], bf16, tag="xw", name="xw_kv0")
            xw0_ap = (xT.ap()[:, 0:512]
                      .rearrange("(kb p) n -> p kb n", p=P))
            for kb in range(KB):
                nc.sync.dma_start(wk_sb[:, kb, :], wk_ap[:, kb, :])
                nc.sync.dma_start(xw0[:, kb, :], xw0_ap[:, kb, :])
            nc.sync.dma_start(
                wq_sb[:], wqT.ap().rearrange("(kb p) d -> p kb d", p=P))
            for kb in range(KB):
                nc.sync.dma_start(wv_sb[:, kb, :], wv_ap[:, kb, :])
            xw_boxes[0][0] = xw0
            # prefetch x windows 1-3 (xw pool has 6 slots: w0..w3, q0, q1)
            for w in range(1, NCH):
                xw_boxes[w][0] = load_xw(w, f"kv{w}")

            # serial: kT(w0, db0) and q0(db0) unblock gstep 0
            for t in kT_db_thunks(xw_boxes[0], 0, 0):
                t()
            qt0 = qT_pool.tile([P, DB, 512], bf16, tag="qTc", name="qT0")
            qT_tiles[0] = qt0
            xwq0_box = [None]
            xwq0_box[0] = load_xw(0, "q0")
            for t in q_db_thunks(0, xwq0_box, qt0, 0):
                t()
            # everything else is deadline-scheduled: kT(w,db) first needed
            # by S(p=db, g=4w) at gstep 16db+4w; v(m) by av at gstep m+3;
            # q0(db) by S(p=db) at 16db
            for w in range(NCH):
                for db in range(DB):
                    if (w, db) == (0, 0):
                        continue
                    bg_schedule(0, 16 * db + 4 * w,
                                kT_db_thunks(xw_boxes[w], w, db))
                for mc in range(4):
                    bg_schedule(0, 4 * w + mc + 2,
                                v_m_thunks(xw_boxes[w], w, mc))
            for db in range(1, DB):
                bg_schedule(0, 16 * db, q_db_thunks(0, xwq0_box, qt0, db))

            expS_scope = tc.tile_pool(name="expS", bufs=5)
            expS_pool = expS_scope.__enter__()
            at_scope = tc.tile_pool(name="at", bufs=2)
            at_pool = at_scope.__enter__()
            small_scope = tc.tile_pool(name="small", bufs=1)
            small_pool = small_scope.__enter__()
            out_scope = tc.tile_pool(name="outsb", bufs=2)
            out_pool = out_scope.__enter__()

            # ---- flat gstep stream: (chunk j, head-pair p, m-tile g) ----
            gsteps = [(j, p, g) for j in range(NCH) for p in range(DB)
                      for g in range(MT)]
            CHUNK = DB * MT
            at_tiles = [None] * NCH
            eS_q = {}
            po_pairs = {}
            pending_p1 = []
            pending_rcp = []
            pending_ep = []

            def emit_S(j, p, g):
                S = ps_S.tile([P, 1024], f32, tag="S", name=f"S{p}_{g}")
                qt = qT_tiles[j]
                nc.tensor.matmul(S[:, 0:512],
                                 lhsT=kT_sb[0:HD, p, g * P:(g + 1) * P],
                                 rhs=qt[0:HD, p, :],
                                 start=True, stop=True)
                nc.tensor.matmul(S[:, 512:1024],
                                 lhsT=kT_sb[HD:P, p, g * P:(g + 1) * P],
                                 rhs=qt[HD:P, p, :],
                                 start=True, stop=True)
                return S

            def emit_av(idx2):
                j, p, g = gsteps[idx2]
                eSp = eS_q.pop((j, p, g))
                key = (j, p)
                if key not in po_pairs:
                    po_pairs[key] = [
                        ps_o.tile([P, 512], f32, tag="o",
                                  name=f"po{j}_{p}_{h}")
                        for h in range(2)]
                po0, po1 = po_pairs[key]
                nc.tensor.matmul(po0[0:HD + 1, :],
                                 lhsT=v_sb[:, g, 2 * p, :],
                                 rhs=eSp[:, 0:512],
                                 start=(g == 0), stop=(g == MT - 1))
                nc.tensor.matmul(po1[0:HD + 1, :],
                                 lhsT=v_sb[:, g, 2 * p + 1, :],
                                 rhs=eSp[:, 512:1024],
                                 start=(g == 0), stop=(g == MT - 1))
                if g == MT - 1:
                    pending_p1.append([3, po0, po1, j, p])

            def emit_epilogue_p1(po0, po1, j, p):
                """Free the po banks; stage denominator rows via DMA."""
                oT0 = small_pool.tile([HD + 1, 512], f32, tag="oT0",
                                      name="oT0")
                oT1 = small_pool.tile([HD + 1, 512], f32, tag="oT1",
                                      name="oT1")
                nc.vector.tensor_copy(out=oT0[:], in_=po0[0:HD + 1, :])
                nc.vector.tensor_copy(out=oT1[:], in_=po1[0:HD + 1, :])
                nc.sync.dma_start(den_sb[0:1, :], oT0[HD:HD + 1, :])
                nc.sync.dma_start(den_sb[32:33, :], oT1[HD:HD + 1, :])
                if debug and (j, p) == (0, 0):
                    nc.sync.dma_start(dbg_po.ap()[:, 0:512], oT0[:])
                    nc.sync.dma_start(dbg_po.ap()[:, 512:1024], oT1[:])
                return oT0, oT1

            def emit_epilogue_recip():
                rcp = small_pool.tile([33, 512], f32r, tag="rcp", name="rcp")
                with nc.allow_low_precision(reason="softmax recip to f32r"):
                    nc.vector.reciprocal(rcp[:], den_sb[:])
                return rcp

            def emit_epilogue_p2(j, p, oT0, oT1, rcp):
                at = at_tiles[j]
                with nc.allow_low_precision(reason="softmax normalize bf16"):
                    bcp0 = ps_bg.tile([P, 512], f32, tag="bg", name="bcp0")
                    nc.tensor.matmul(bcp0[0:HD, :],
                                     lhsT=ones_r[0:1, :],
                                     rhs=rcp[0:1, :],
                                     start=True, stop=True)
                    nc.vector.tensor_tensor(
                        out=at[0:HD, p, :], in0=oT0[0:HD, :],
                        in1=bcp0[0:HD, :], op=MULT)
                    bcp1 = ps_bg.tile([P, 512], f32, tag="bg", name="bcp1")
                    nc.tensor.matmul(bcp1[0:HD, :],
                                     lhsT=ones_r[32:33, :],
                                     rhs=rcp[32:33, :],
                                     start=True, stop=True)
                    tmp1 = small_pool.tile([HD, 512], bf16, tag="tmp1",
                                           name="tmp1")
                    nc.vector.tensor_tensor(
                        out=tmp1[:], in0=oT1[0:HD, :],
                        in1=bcp1[0:HD, :], op=MULT)
                    nc.sync.dma_start(at[HD:P, p, :], tmp1[:])

            def drain_queues():
                # later stages first: den_sb is shared, so pair n's recip
                # must be emitted before pair n+1's den DMAs
                for ep in pending_ep:
                    ep[0] -= 1
                while pending_ep and pending_ep[0][0] <= 0:
                    _, j, p, oT0, oT1, rcp = pending_ep.pop(0)
                    emit_epilogue_p2(j, p, oT0, oT1, rcp)
                for ep in pending_rcp:
                    ep[0] -= 1
                while pending_rcp and pending_rcp[0][0] <= 0:
                    _, j, p, oT0, oT1 = pending_rcp.pop(0)
                    rcp = emit_epilogue_recip()
                    # bcp matmuls enter the in-order PE queue only after
                    # the 3.3us batched DVE reciprocal has surely finished
                    pending_ep.append([5, j, p, oT0, oT1, rcp])
                for ep in pending_p1:
                    ep[0] -= 1
                while pending_p1 and pending_p1[0][0] <= 0:
                    _, po0, po1, j, p = pending_p1.pop(0)
                    oT0, oT1 = emit_epilogue_p1(po0, po1, j, p)
                    pending_rcp.append([2, j, p, oT0, oT1])

            def start_chunk(c, idx):
                if c == 0:
                    wp_scope = tc.tile_pool(name="wp", bufs=1)
                    wp_pool = wp_scope.__enter__()
                    wp_box.append(wp_scope)  # keep scope alive
                    wp_sb = wp_pool.tile([P, DB, D], bf16, tag="wp")
                    nc.sync.dma_start(
                        wp_sb[:],
                        wpT.ap().rearrange("(cb p) e -> p cb e", p=P))
                    wp_box[0] = wp_sb
                if debug and c == 1:
                    nc.sync.dma_start(dbg_kT.ap(), kT_sb[:])
                    nc.sync.dma_start(dbg_v.ap(), v_sb[:])
                    nc.sync.dma_start(dbg_qT.ap(), qT_tiles[0][:])
                if debug and c == 2:
                    nc.sync.dma_start(dbg_at.ap(), at_tiles[0][:])
                at_tiles[c] = at_pool.tile([P, DB, 512], bf16, tag="at",
                                           name=f"at{c}")
                if c + 1 < NCH:
                    schedule_qT(c + 1, idx, idx + CHUNK)
                if c >= 1:
                    bg_schedule(idx + 16, idx + CHUNK,
                                emit_proj_thunks(c - 1))

            start_chunk(0, 0)
            S_next = emit_S(*gsteps[0])
            for idx, (j, p, g) in enumerate(gsteps):
                S_cur = S_next
                eS = expS_pool.tile([P, 1024], bf16, tag="e",
                                    name=f"eS{p}_{g}")
                nc.scalar.activation(eS[:], S_cur[:], Exp, scale=SCALE)
                if debug and idx == 0:
                    nc.sync.dma_start(dbg_eS.ap(), eS[:])
                eS_q[(j, p, g)] = eS
                if idx >= AV_LAG:
                    emit_av(idx - AV_LAG)
                drain_queues()
                if idx + 1 < len(gsteps):
                    if (idx + 1) % CHUNK == 0:
                        bg_drain_all(idx)
                        start_chunk((idx + 1) // CHUNK, idx + 1)
                    bg_tick(idx, idx + 1)
                    S_next = emit_S(*gsteps[idx + 1])

            for idx2 in range(len(gsteps) - AV_LAG, len(gsteps)):
                emit_av(idx2)
            while pending_p1 or pending_rcp or pending_ep:
                drain_queues()
            bg_drain_all(10 ** 9)

            # final chunk's projection
            for t in emit_proj_thunks(NCH - 1):
                t()

            if len(wp_box) > 1:
                wp_box[1].__exit__(None, None, None)
            out_scope.__exit__(None, None, None)
            small_scope.__exit__(None, None, None)
            at_scope.__exit__(None, None, None)
            expS_scope.__exit__(None, None, None)
            wkv_scope.__exit__(None, None, None)

    nc.compile()
    return nc


_CACHE: dict = {}


def _get_program():
    if "nc" not in _CACHE:
        _CACHE["nc"] = build_program()
    return _CACHE["nc"]


def make_in_maps(x, w_qkv, w_proj):
    """Host-side sharding: per-core input dict (bf16)."""
    bf = ml_dtypes.bfloat16
    x = np.asarray(x, dtype=np.float32)
    w_qkv = np.asarray(w_qkv, dtype=np.float32)
    w_proj = np.asarray(w_proj, dtype=np.float32)
    in_maps = []
    for core in range(8):
        b, g = divmod(core, 2)
        gsl = slice(g * DC, (g + 1) * DC)
        in_maps.append({
            "xT": np.ascontiguousarray(x[b].T.astype(bf)),            # [D, N]
            "wqT": np.ascontiguousarray(w_qkv[0 * D:1 * D][gsl].T.astype(bf)),
            "wkT": np.ascontiguousarray(w_qkv[1 * D:2 * D][gsl].T.astype(bf)),
            "wvT": np.ascontiguousarray(w_qkv[2 * D:3 * D][gsl].T.astype(bf)),
            "wpT": np.ascontiguousarray(w_proj[:, gsl].T.astype(bf)),
        })
    return in_maps


def run(x, w_qkv, w_proj, b_proj, **spmd_kwargs):
    nc = _get_program()
    in_maps = make_in_maps(x, w_qkv, w_proj)
    res = run_bass_kernel_spmd(nc, in_maps, list(range(8)), **spmd_kwargs)
    b_proj = np.asarray(b_proj, dtype=np.float32)
    outp = np.empty((B, N, D), dtype=np.float32)
    for b in range(B):
        outp[b] = (res.results[2 * b]["out"] + res.results[2 * b + 1]["out"]
                   + b_proj[None, :])
    return outp, res


def kernel(x, w_qkv, w_proj, b_proj):
    outp, _ = run(x, w_qkv, w_proj, b_proj)
    return outp


# revision 23
# speedup vs baseline: 1.1349x; 1.0392x over previous
"""Multi-head attention (B=4, N=2048, D=1024, H=16) on 8 Trainium2 NeuronCores.

Sharding: core = (batch b = core//2, head-group g = core%2 of 8 heads).
Each core computes qkv + attention for its 8 heads and a *partial* output
projection over its 512 features; the host sums the two partials per batch
and adds the bias (the tensor-parallel unshard).

All matmul operands are bf16 (inputs cast on host). Score matmuls exploit
PE sub-array tiling: per (head-pair p, m-tile g) TWO row-tiled matmuls run
CONCURRENTLY on disjoint halves of the 128x128 array (head 2p contracts on
rows 0-63, head 2p+1 on rows 64-127; K=64 each, no zero-padded q). One
[128,1024] PSUM tile holds S^T for both heads (512 n-cols each), so exp
shape/count is unchanged but the PE spends half the cycles on scores.

attn@v keeps the ones-column-in-v trick (out rows 65) for softmax
denominators; h0/h1 accumulate in parallel po banks (ps_o bufs=2).

Emission is a flat gstep stream (chunk j, pair p, m-tile g) paced for the
ACT engine (exp is the roofline: 33.5M elements at 1/lane/cycle @1.2GHz =
293us). All qkv/projection work runs as EDF-scheduled background thunks:
each kT window/pair-block, v m-tile, and q block carries the gstep index
deadline at which the foreground stream first needs it, so the first exp
fires ~10us in and k/v production for windows 1-3 overlaps chunk 0's exp
stream. Pair epilogues are staged (oT copies at +3, denominator DMAs into
a [33,512] staging tile + ONE batched DVE reciprocal at +5, ones-broadcast
bcp matmuls + normalize multiplies at +10) so the 3.3us reciprocal never
head-of-line-blocks the in-order PE queue.
"""
import heapq
import sys

sys.path.insert(0, '/opt/trn_rl_repo')

import numpy as np
import ml_dtypes

import concourse.bass as bass  # noqa: F401  (registers engines)
import concourse.mybir as mybir
import concourse.tile as tile
from concourse import bacc
from concourse.bass_utils import run_bass_kernel_spmd

dt = mybir.dt

B = 4
N = 2048          # sequence length
D = 1024          # d_model
NH = 16           # total heads
HD = 64           # head dim
NHC = 8           # heads per core
DC = NHC * HD     # 512 features per core
SCALE = HD ** -0.5

P = 128           # partitions
KB = D // P       # 8 k-blocks
NCH = N // 512    # 4 n-chunks of 512
MT = N // P       # 16 m-tiles of 128
DB = DC // P      # 4 head-pair blocks

AV_LAG = 3
PACE = 1          # paced background thunks per gstep beyond deadline-forced


def build_program(debug=False):
    nc = bacc.Bacc("TRN2", target_bir_lowering=False, debug=False,
                   enable_asserts=False, num_devices=8)

    bf16 = dt.bfloat16
    f32 = dt.float32
    f32r = dt.float32r
    Exp = mybir.ActivationFunctionType.Exp
    MULT = mybir.AluOpType.mult

    xT = nc.dram_tensor("xT", [D, N], bf16, kind="ExternalInput")
    wqT = nc.dram_tensor("wqT", [D, DC], bf16, kind="ExternalInput")
    wkT = nc.dram_tensor("wkT", [D, DC], bf16, kind="ExternalInput")
    wvT = nc.dram_tensor("wvT", [D, DC], bf16, kind="ExternalInput")
    wpT = nc.dram_tensor("wpT", [DC, D], bf16, kind="ExternalInput")
    out = nc.dram_tensor("out", [N, D], f32, kind="ExternalOutput")
    if debug:
        dbg_kT = nc.dram_tensor("dbg_kT", [P, DB, N], bf16,
                                kind="ExternalOutput")
        dbg_qT = nc.dram_tensor("dbg_qT", [P, DB, 512], bf16,
                                kind="ExternalOutput")
        dbg_v = nc.dram_tensor("dbg_v", [P, MT, NHC, HD + 1], bf16,
                               kind="ExternalOutput")
        dbg_eS = nc.dram_tensor("dbg_eS", [P, 1024], bf16,
                                kind="ExternalOutput")
        dbg_po = nc.dram_tensor("dbg_po", [HD + 1, 1024], f32,
                                kind="ExternalOutput")
        dbg_at = nc.dram_tensor("dbg_at", [P, DB, 512], bf16,
                                kind="ExternalOutput")

    with tile.TileContext(nc) as tc:
        with tc.tile_pool(name="persist", bufs=1) as persist, \
             tc.tile_pool(name="wq", bufs=1) as wq_pool, \
             tc.tile_pool(name="qTc", bufs=2) as qT_pool, \
             tc.tile_pool(name="xw", bufs=4) as xw_pool, \
             tc.tile_pool(name="ps_S", bufs=2, space="PSUM") as ps_S, \
             tc.tile_pool(name="ps_bg", bufs=2, space="PSUM") as ps_bg, \
             tc.tile_pool(name="ps_o", bufs=2, space="PSUM") as ps_o:

            # ---- persistent SBUF tensors ----
            kT_sb = persist.tile([P, DB, N], bf16, tag="kT")
            # v with a ones column per head: [m-part, m-tile, head, 65]
            v_sb = persist.tile([P, MT, NHC, HD + 1], bf16, tag="v")
            ones_r = persist.tile([P, HD], f32r, tag="ones")
            # denominator staging: pair's two softmax-denominator rows land
            # on partitions 0 and 32; rows 1-31 stay 1.0 so one batched
            # reciprocal covers both heads
            den_sb = persist.tile([33, 512], f32, tag="den")

            wq_sb = wq_pool.tile([P, KB, DC], bf16, tag="wq")
            nc.vector.memset(v_sb[:], 1.0)
            nc.vector.memset(ones_r[:].bitcast(f32), 1.0)
            nc.vector.memset(den_sb[:], 1.0)

            def load_xw(j, label, fine=False):
                xw = xw_pool.tile([P, KB, 512], bf16, tag="xw",
                                  name=f"xw_{label}")
                ap = (xT.ap()[:, j * 512:(j + 1) * 512]
                      .rearrange("(kb p) n -> p kb n", p=P))
                if fine:
                    for kb in range(KB):
                        nc.sync.dma_start(xw[:, kb, :], ap[:, kb, :])
                else:
                    nc.sync.dma_start(xw[:], ap)
                return xw

            # ---- background thunk machinery (EDF) ----
            bg_wait = []          # appended in nondecreasing avail order

            bg_heap = []          # (deadline, seq, thunk)
            bg_seq = [0]

            def bg_schedule(avail, deadline, thunks):
                for t in thunks:
                    bg_wait.append((avail, deadline, bg_seq[0], t))
                    bg_seq[0] += 1

            def bg_tick(idx, next_idx, pace=PACE):
                while bg_wait and bg_wait[0][0] <= idx:
                    _, dl, seq, t = bg_wait.pop(0)
                    heapq.heappush(bg_heap, (dl, seq, t))
                while bg_heap and bg_heap[0][0] <= next_idx:
                    heapq.heappop(bg_heap)[2]()
                for _ in range(pace):
                    if bg_heap:
                        heapq.heappop(bg_heap)[2]()

            def bg_drain_all(idx):
                while bg_wait and bg_wait[0][0] <= idx:
                    _, dl, seq, t = bg_wait.pop(0)
                    heapq.heappush(bg_heap, (dl, seq, t))
                while bg_heap:
                    heapq.heappop(bg_heap)[2]()

            # ---- qkv production thunks ----
            def kT_db_thunks(xw_box, w, db):
                """k projection block (window w, pair db): 4 mm + 1 copy."""
                box = [None]
                thunks = []

                def mm_t(kb0):
                    if kb0 == 0:
                        box[0] = ps_bg.tile([P, 512], f32, tag="bg",
                                            name=f"pk{w}_{db}")
                    for kb in (kb0, kb0 + 1):
                        nc.tensor.matmul(
                            box[0][:],
                            lhsT=wk_box[0][:, kb, db * P:(db + 1) * P],
                            rhs=xw_box[0][:, kb, :],
                            start=(kb == 0), stop=(kb == KB - 1))
                for kb0 in range(0, KB, 2):
                    thunks.append(lambda kb0=kb0: mm_t(kb0))

                def cp_t():
                    nc.vector.tensor_copy(
                        out=kT_sb[:, db, w * 512:(w + 1) * 512],
                        in_=box[0][:])
                thunks.append(cp_t)
                return thunks

            def v_m_thunks(xw_box, w, mc):
                """v for m-tile 4w+mc: 4 mm + 1 copy."""
                m = w * 4 + mc
                box = [None]
                thunks = []

                def mm_t(kb0):
                    if kb0 == 0:
                        box[0] = ps_bg.tile([P, 512], f32, tag="bg",
                                            name=f"pv{m}")
                    for kb in (kb0, kb0 + 1):
                        nc.tensor.matmul(
                            box[0][:],
                            lhsT=xw_box[0][:, kb, mc * P:(mc + 1) * P],
                            rhs=wv_box[0][:, kb, :],
                            start=(kb == 0), stop=(kb == KB - 1))
                for kb0 in range(0, KB, 2):
                    thunks.append(lambda kb0=kb0: mm_t(kb0))

                def cp_t():
                    nc.vector.tensor_copy(
                        out=v_sb[:, m, :, 0:HD],
                        in_=box[0][:].rearrange("p (h d) -> p h d", h=NHC))
                thunks.append(cp_t)
                return thunks

            qT_tiles = [None] * NCH

            def q_db_thunks(jn, xw_box, qt, db):
                """q block db of chunk jn: 4 mm + 1 copy (no zero pad)."""
                box = [None]
                thunks = []

                def mm_t(kb0):
                    if kb0 == 0:
                        box[0] = ps_bg.tile([P, 512], f32, tag="bg",
                                            name=f"pq{jn}_{db}")
                    for kb in (kb0, kb0 + 1):
                        nc.tensor.matmul(
                            box[0][:],
                            lhsT=wq_sb[:, kb, db * P:(db + 1) * P],
                            rhs=xw_box[0][:, kb, :],
                            start=(kb == 0), stop=(kb == KB - 1))
                for kb0 in range(0, KB, 2):
                    thunks.append(lambda kb0=kb0: mm_t(kb0))

                def cp_t():
                    nc.vector.tensor_copy(out=qt[:, db, :], in_=box[0][:])
                thunks.append(cp_t)
                return thunks

            def schedule_qT(jn, avail, deadline):
                qt = qT_pool.tile([P, DB, 512], bf16, tag="qTc",
                                  name=f"qT{jn}")
                qT_tiles[jn] = qt
                thunks = []
                for db in range(DB):
                    thunks.extend(q_db_thunks(jn, xw_boxes[jn], qt, db))
                bg_schedule(avail, deadline, thunks)

            def emit_proj_thunks(j):
                """Projection of chunk j (at_j is bf16)."""
                at_j = at_tiles[j]
                thunks = []
                box = [None]
                for ns in range(4):
                    for ec in range(2):
                        def mm_t(ns, ec, cb0):
                            if cb0 == 0:
                                box[0] = ps_bg.tile([P, 512], f32, tag="bg",
                                                    name=f"pp{ns}_{ec}")
                            for cb in (cb0, cb0 + 1):
                                nc.tensor.matmul(
                                    box[0][:],
                                    lhsT=at_j[:, cb, ns * P:(ns + 1) * P],
                                    rhs=wp_box[0][:, cb,
                                                  ec * 512:(ec + 1) * 512],
                                    start=(cb == 0), stop=(cb == DB - 1))
                        for cb0 in range(0, DB, 2):
                            thunks.append(
                                lambda ns=ns, ec=ec, cb0=cb0: mm_t(ns, ec, cb0))
                        def cp_t(ns=ns, ec=ec):
                            osb = out_pool.tile([P, 512], f32, tag="osb",
                                                name=f"osb{ns}_{ec}")
                            nc.vector.tensor_copy(out=osb[:], in_=box[0][:])
                            nc.sync.dma_start(
                                out.ap()[j * 512 + ns * P:
                                         j * 512 + (ns + 1) * P,
                                         ec * 512:(ec + 1) * 512],
                                osb[:])
                        thunks.append(cp_t)
                return thunks

            wk_box = [None]
            wv_box = [None]
            wp_box = [None]
            xw_boxes = [[None] for _ in range(NCH)]

            # ---- prelude: DMAs + minimal serial work for gstep 0 ----
            wkv_scope = tc.tile_pool(name="wkv", bufs=1)
            wkv_pool = wkv_scope.__enter__()
            wk_sb = wkv_pool.tile([P, KB, DC], bf16, tag="wk")
            wv_sb = wkv_pool.tile([P, KB, DC], bf16, tag="wv")
            wk_box[0] = wk_sb
            wv_box[0] = wv_sb
            wk_ap = wkT.ap().rearrange("(kb p) d -> p kb d", p=P)
            wv_ap = wvT.ap().rearrange("(kb p) d -> p kb d", p=P)
            xw0 = xw_pool.tile([P, KB, 512], bf16, tag="xw", name="xw_kv0")
            xw0_ap = (xT.ap()[:, 0:512]
                      .rearrange("(kb p) n -> p kb n", p=P))
            wq_ap = wqT.ap().rearrange("(kb p) d -> p kb d", p=P)
            for kb in range(KB):
                nc.sync.dma_start(wk_sb[:, kb, :], wk_ap[:, kb, :])
                nc.sync.dma_start(xw0[:, kb, :], xw0_ap[:, kb, :])
                nc.sync.dma_start(wq_sb[:, kb, :], wq_ap[:, kb, :])
            for kb in range(KB):
                nc.sync.dma_start(wv_sb[:, kb, :], wv_ap[:, kb, :])
            xw_boxes[0][0] = xw0
            # prefetch x windows 1-3; kv and qT production share these
            # tiles (x is DMA'd once per window)
            for w in range(1, NCH):
                xw_boxes[w][0] = load_xw(w, f"kv{w}", fine=True)

            # serial: kT(w0, db0) and q0(db0) unblock gstep 0
            for t in kT_db_thunks(xw_boxes[0], 0, 0):
                t()
            qt0 = qT_pool.tile([P, DB, 512], bf16, tag="qTc", name="qT0")
            qT_tiles[0] = qt0
            for t in q_db_thunks(0, xw_boxes[0], qt0, 0):
                t()
            # everything else is deadline-scheduled: kT(w,db) first needed
            # by S(p=db, g=4w) at gstep 16db+4w; v(m) by av at gstep m+3;
            # q0(db) by S(p=db) at 16db
            for w in range(NCH):
                for db in range(DB):
                    if (w, db) == (0, 0):
                        continue
                    bg_schedule(0, 16 * db + 4 * w,
                                kT_db_thunks(xw_boxes[w], w, db))
                for mc in range(4):
                    bg_schedule(0, 4 * w + mc + 2,
                                v_m_thunks(xw_boxes[w], w, mc))
            for db in range(1, DB):
                bg_schedule(0, 16 * db, q_db_thunks(0, xw_boxes[0], qt0, db))

            expS_scope = tc.tile_pool(name="expS", bufs=5)
            expS_pool = expS_scope.__enter__()
            at_scope = tc.tile_pool(name="at", bufs=2)
            at_pool = at_scope.__enter__()
            small_scope = tc.tile_pool(name="small", bufs=1)
            small_pool = small_scope.__enter__()
            out_scope = tc.tile_pool(name="outsb", bufs=2)
            out_pool = out_scope.__enter__()

            # ---- flat gstep stream: (chunk j, head-pair p, m-tile g) ----
            gsteps = [(j, p, g) for j in range(NCH) for p in range(DB)
                      for g in range(MT)]
            CHUNK = DB * MT
            at_tiles = [None] * NCH
            eS_q = {}
            po_pairs = {}
            pending_p1 = []
            pending_rcp = []
            pending_ep = []

            def emit_S(j, p, g):
                S = ps_S.tile([P, 1024], f32, tag="S", name=f"S{p}_{g}")
                qt = qT_tiles[j]
                nc.tensor.matmul(S[:, 0:512],
                                 lhsT=kT_sb[0:HD, p, g * P:(g + 1) * P],
                                 rhs=qt[0:HD, p, :],
                                 start=True, stop=True)
                nc.tensor.matmul(S[:, 512:1024],
                                 lhsT=kT_sb[HD:P, p, g * P:(g + 1) * P],
                                 rhs=qt[HD:P, p, :],
                                 start=True, stop=True)
                return S

            def emit_av(idx2):
                j, p, g = gsteps[idx2]
                eSp = eS_q.pop((j, p, g))
                key = (j, p)
                if key not in po_pairs:
                    po_pairs[key] = [
                        ps_o.tile([P, 512], f32, tag="o",
                                  name=f"po{j}_{p}_{h}")
                        for h in range(2)]
                po0, po1 = po_pairs[key]
                nc.tensor.matmul(po0[0:HD + 1, :],
                                 lhsT=v_sb[:, g, 2 * p, :],
                                 rhs=eSp[:, 0:512],
                                 start=(g == 0), stop=(g == MT - 1))
                nc.tensor.matmul(po1[0:HD + 1, :],
                                 lhsT=v_sb[:, g, 2 * p + 1, :],
                                 rhs=eSp[:, 512:1024],
                                 start=(g == 0), stop=(g == MT - 1))
                if g == MT - 1:
                    pending_p1.append([3, po0, po1, j, p])

            def emit_epilogue_p1(po0, po1, j, p):
                """Free the po banks; stage denominator rows via DMA."""
                oT0 = small_pool.tile([HD + 1, 512], f32, tag="oT0",
                                      name="oT0")
                oT1 = small_pool.tile([HD + 1, 512], f32, tag="oT1",
                                      name="oT1")
                nc.vector.tensor_copy(out=oT0[:], in_=po0[0:HD + 1, :])
                nc.vector.tensor_copy(out=oT1[:], in_=po1[0:HD + 1, :])
                nc.sync.dma_start(den_sb[0:1, :], oT0[HD:HD + 1, :])
                nc.sync.dma_start(den_sb[32:33, :], oT1[HD:HD + 1, :])
                if debug and (j, p) == (0, 0):
                    nc.sync.dma_start(dbg_po.ap()[:, 0:512], oT0[:])
                    nc.sync.dma_start(dbg_po.ap()[:, 512:1024], oT1[:])
                return oT0, oT1

            def emit_epilogue_recip():
                rcp = small_pool.tile([33, 512], f32r, tag="rcp", name="rcp")
                with nc.allow_low_precision(reason="softmax recip to f32r"):
                    nc.vector.reciprocal(rcp[:], den_sb[:])
                return rcp

            def emit_epilogue_p2(j, p, oT0, oT1, rcp):
                at = at_tiles[j]
                with nc.allow_low_precision(reason="softmax normalize bf16"):
                    bcp0 = ps_bg.tile([P, 512], f32, tag="bg", name="bcp0")
                    nc.tensor.matmul(bcp0[0:HD, :],
                                     lhsT=ones_r[0:1, :],
                                     rhs=rcp[0:1, :],
                                     start=True, stop=True)
                    nc.vector.tensor_tensor(
                        out=at[0:HD, p, :], in0=oT0[0:HD, :],
                        in1=bcp0[0:HD, :], op=MULT)
                    bcp1 = ps_bg.tile([P, 512], f32, tag="bg", name="bcp1")
                    nc.tensor.matmul(bcp1[0:HD, :],
                                     lhsT=ones_r[32:33, :],
                                     rhs=rcp[32:33, :],
                                     start=True, stop=True)
                    tmp1 = small_pool.tile([HD, 512], bf16, tag="tmp1",
                                           name="tmp1")
                    nc.vector.tensor_tensor(
                        out=tmp1[:], in0=oT1[0:HD, :],
                        in1=bcp1[0:HD, :], op=MULT)
                    nc.sync.dma_start(at[HD:P, p, :], tmp1[:])

            def drain_queues():
                # later stages first: den_sb is shared, so pair n's recip
                # must be emitted before pair n+1's den DMAs
                for ep in pending_ep:
                    ep[0] -= 1
                while pending_ep and pending_ep[0][0] <= 0:
                    _, j, p, oT0, oT1, rcp = pending_ep.pop(0)
                    emit_epilogue_p2(j, p, oT0, oT1, rcp)
                for ep in pending_rcp:
                    ep[0] -= 1
                while pending_rcp and pending_rcp[0][0] <= 0:
                    _, j, p, oT0, oT1 = pending_rcp.pop(0)
                    rcp = emit_epilogue_recip()
                    # bcp matmuls enter the in-order PE queue only after
                    # the 3.3us batched DVE reciprocal has surely finished
                    pending_ep.append([7, j, p, oT0, oT1, rcp])
                for ep in pending_p1:
                    ep[0] -= 1
                while pending_p1 and pending_p1[0][0] <= 0:
                    _, po0, po1, j, p = pending_p1.pop(0)
                    oT0, oT1 = emit_epilogue_p1(po0, po1, j, p)
                    pending_rcp.append([2, j, p, oT0, oT1])

            def start_chunk(c, idx):
                if c == 0:
                    wp_scope = tc.tile_pool(name="wp", bufs=1)
                    wp_pool = wp_scope.__enter__()
                    wp_box.append(wp_scope)  # keep scope alive
                    wp_sb = wp_pool.tile([P, DB, D], bf16, tag="wp")
                    wp_ap = wpT.ap().rearrange("(cb p) e -> p cb e", p=P)
                    for cb in range(DB):
                        nc.sync.dma_start(wp_sb[:, cb, :], wp_ap[:, cb, :])
                    wp_box[0] = wp_sb
                if debug and c == 1:
                    nc.sync.dma_start(dbg_kT.ap(), kT_sb[:])
                    nc.sync.dma_start(dbg_v.ap(), v_sb[:])
                    nc.sync.dma_start(dbg_qT.ap(), qT_tiles[0][:])
                if debug and c == 2:
                    nc.sync.dma_start(dbg_at.ap(), at_tiles[0][:])
                at_tiles[c] = at_pool.tile([P, DB, 512], bf16, tag="at",
                                           name=f"at{c}")
                if c + 1 < NCH:
                    schedule_qT(c + 1, idx, idx + CHUNK)
                if c >= 1:
                    bg_schedule(idx + 16, idx + CHUNK,
                                emit_proj_thunks(c - 1))

            start_chunk(0, 0)
            S_next = emit_S(*gsteps[0])
            for idx, (j, p, g) in enumerate(gsteps):
                S_cur = S_next
                eS = expS_pool.tile([P, 1024], bf16, tag="e",
                                    name=f"eS{p}_{g}")
                nc.scalar.activation(eS[:], S_cur[:], Exp, scale=SCALE)
                if debug and idx == 0:
                    nc.sync.dma_start(dbg_eS.ap(), eS[:])
                eS_q[(j, p, g)] = eS
                if idx >= AV_LAG:
                    emit_av(idx - AV_LAG)
                drain_queues()
                if idx + 1 < len(gsteps):
                    if (idx + 1) % CHUNK == 0:
                        bg_drain_all(idx)
                        start_chunk((idx + 1) // CHUNK, idx + 1)
                    bg_tick(idx, idx + 1)
                    S_next = emit_S(*gsteps[idx + 1])

            for idx2 in range(len(gsteps) - AV_LAG, len(gsteps)):
                emit_av(idx2)
            while pending_p1 or pending_rcp or pending_ep:
                drain_queues()
            bg_drain_all(10 ** 9)

            # final chunk's projection
            for t in emit_proj_thunks(NCH - 1):
                t()

            if len(wp_box) > 1:
                wp_box[1].__exit__(None, None, None)
            out_scope.__exit__(None, None, None)
            small_scope.__exit__(None, None, None)
            at_scope.__exit__(None, None, None)
            expS_scope.__exit__(None, None, None)
            wkv_scope.__exit__(None, None, None)

    nc.compile()
    return nc


_CACHE: dict = {}


def _get_program():
    if "nc" not in _CACHE:
        _CACHE["nc"] = build_program()
    return _CACHE["nc"]


def make_in_maps(x, w_qkv, w_proj):
    """Host-side sharding: per-core input dict (bf16)."""
    bf = ml_dtypes.bfloat16
    x = np.asarray(x, dtype=np.float32)
    w_qkv = np.asarray(w_qkv, dtype=np.float32)
    w_proj = np.asarray(w_proj, dtype=np.float32)
    in_maps = []
    for core in range(8):
        b, g = divmod(core, 2)
        gsl = slice(g * DC, (g + 1) * DC)
        in_maps.append({
            "xT": np.ascontiguousarray(x[b].T.astype(bf)),            # [D, N]
            "wqT": np.ascontiguousarray(w_qkv[0 * D:1 * D][gsl].T.astype(bf)),
            "wkT": np.ascontiguousarray(w_qkv[1 * D:2 * D][gsl].T.astype(bf)),
            "wvT": np.ascontiguousarray(w_qkv[2 * D:3 * D][gsl].T.astype(bf)),
            "wpT": np.ascontiguousarray(w_proj[:, gsl].T.astype(bf)),
        })
    return in_maps


def run(x, w_qkv, w_proj, b_proj, **spmd_kwargs):
    nc = _get_program()
    in_maps = make_in_maps(x, w_qkv, w_proj)
    res = run_bass_kernel_spmd(nc, in_maps, list(range(8)), **spmd_kwargs)
    b_proj = np.asarray(b_proj, dtype=np.float32)
    outp = np.empty((B, N, D), dtype=np.float32)
    for b in range(B):
        outp[b] = (res.results[2 * b]["out"] + res.results[2 * b + 1]["out"]
                   + b_proj[None, :])
    return outp, res


def kernel(x, w_qkv, w_proj, b_proj):
    outp, _ = run(x, w_qkv, w_proj, b_proj)
    return outp


# revision 24
# speedup vs baseline: 1.1469x; 1.0106x over previous
"""Multi-head attention (B=4, N=2048, D=1024, H=16) on 8 Trainium2 NeuronCores.

Sharding: core = (batch b = core//2, head-group g = core%2 of 8 heads).
Each core computes qkv + attention for its 8 heads and a *partial* output
projection over its 512 features; the host sums the two partials per batch
and adds the bias (the tensor-parallel unshard).

All matmul operands are bf16 (inputs cast on host). Score matmuls exploit
PE sub-array tiling: per (head-pair p, m-tile g) TWO row-tiled matmuls run
CONCURRENTLY on disjoint halves of the 128x128 array (head 2p contracts on
rows 0-63, head 2p+1 on rows 64-127; K=64 each, no zero-padded q). One
[128,1024] PSUM tile holds S^T for both heads (512 n-cols each), so exp
shape/count is unchanged but the PE spends half the cycles on scores.

attn@v keeps the ones-column-in-v trick (out rows 65) for softmax
denominators; h0/h1 accumulate in parallel po banks (ps_o bufs=2).

Emission is a flat gstep stream (chunk j, pair p, m-tile g) paced for the
ACT engine (exp is the roofline: 33.5M elements at 1/lane/cycle @1.2GHz =
293us). All qkv/projection work runs as EDF-scheduled background thunks:
each kT window/pair-block, v m-tile, and q block carries the gstep index
deadline at which the foreground stream first needs it, so the first exp
fires ~10us in and k/v production for windows 1-3 overlaps chunk 0's exp
stream. Pair epilogues are staged (oT copies at +3, denominator DMAs into
a [33,512] staging tile + ONE batched DVE reciprocal at +5, ones-broadcast
bcp matmuls + normalize multiplies at +10) so the 3.3us reciprocal never
head-of-line-blocks the in-order PE queue.
"""
import heapq
import sys

sys.path.insert(0, '/opt/trn_rl_repo')

import numpy as np
import ml_dtypes

import concourse.bass as bass  # noqa: F401  (registers engines)
import concourse.mybir as mybir
import concourse.tile as tile
from concourse import bacc
from concourse.bass_utils import run_bass_kernel_spmd

dt = mybir.dt

B = 4
N = 2048          # sequence length
D = 1024          # d_model
NH = 16           # total heads
HD = 64           # head dim
NHC = 8           # heads per core
DC = NHC * HD     # 512 features per core
SCALE = HD ** -0.5

P = 128           # partitions
KB = D // P       # 8 k-blocks
NCH = N // 512    # 4 n-chunks of 512
MT = N // P       # 16 m-tiles of 128
DB = DC // P      # 4 head-pair blocks

AV_LAG = 3
PACE = 1          # paced background thunks per gstep beyond deadline-forced


def build_program(debug=False):
    nc = bacc.Bacc("TRN2", target_bir_lowering=False, debug=False,
                   enable_asserts=False, num_devices=8)

    bf16 = dt.bfloat16
    f32 = dt.float32
    f32r = dt.float32r
    Exp = mybir.ActivationFunctionType.Exp
    MULT = mybir.AluOpType.mult

    # host-prearranged layouts: partition-dim first, contiguous fat lines
    xh = nc.dram_tensor("xh", [P, NCH, KB, 512], bf16, kind="ExternalInput")
    wqh = nc.dram_tensor("wqh", [P, KB, DC], bf16, kind="ExternalInput")
    wkh = nc.dram_tensor("wkh", [P, KB, DC], bf16, kind="ExternalInput")
    wvh = nc.dram_tensor("wvh", [P, KB, DC], bf16, kind="ExternalInput")
    wph = nc.dram_tensor("wph", [P, DB, D], bf16, kind="ExternalInput")
    out = nc.dram_tensor("out", [N, D], f32, kind="ExternalOutput")
    if debug:
        dbg_kT = nc.dram_tensor("dbg_kT", [P, DB, N], bf16,
                                kind="ExternalOutput")
        dbg_qT = nc.dram_tensor("dbg_qT", [P, DB, 512], bf16,
                                kind="ExternalOutput")
        dbg_v = nc.dram_tensor("dbg_v", [P, MT, NHC, HD + 1], bf16,
                               kind="ExternalOutput")
        dbg_eS = nc.dram_tensor("dbg_eS", [P, 1024], bf16,
                                kind="ExternalOutput")
        dbg_po = nc.dram_tensor("dbg_po", [HD + 1, 1024], f32,
                                kind="ExternalOutput")
        dbg_at = nc.dram_tensor("dbg_at", [P, DB, 512], bf16,
                                kind="ExternalOutput")

    with tile.TileContext(nc) as tc:
        with tc.tile_pool(name="persist", bufs=1) as persist, \
             tc.tile_pool(name="wq", bufs=1) as wq_pool, \
             tc.tile_pool(name="qTc", bufs=2) as qT_pool, \
             tc.tile_pool(name="xw", bufs=4) as xw_pool, \
             tc.tile_pool(name="ps_S", bufs=2, space="PSUM") as ps_S, \
             tc.tile_pool(name="ps_bg", bufs=2, space="PSUM") as ps_bg, \
             tc.tile_pool(name="ps_o", bufs=2, space="PSUM") as ps_o:

            # ---- persistent SBUF tensors ----
            kT_sb = persist.tile([P, DB, N], bf16, tag="kT")
            # v with a ones column per head: [m-part, m-tile, head, 65]
            v_sb = persist.tile([P, MT, NHC, HD + 1], bf16, tag="v")
            ones_r = persist.tile([P, HD], f32r, tag="ones")
            # denominator staging: pair's two softmax-denominator rows land
            # on partitions 0 and 32; rows 1-31 stay 1.0 so one batched
            # reciprocal covers both heads
            den_sb = persist.tile([33, 512], f32, tag="den")

            wq_sb = wq_pool.tile([P, KB, DC], bf16, tag="wq")
            nc.vector.memset(v_sb[:], 1.0)
            nc.vector.memset(ones_r[:].bitcast(f32), 1.0)
            nc.vector.memset(den_sb[:], 1.0)

            def load_xw(j, label, fine=False):
                xw = xw_pool.tile([P, KB, 512], bf16, tag="xw",
                                  name=f"xw_{label}")
                ap = xh.ap()[:, j, :, :]
                if fine:
                    for kb in range(KB):
                        nc.sync.dma_start(xw[:, kb, :], ap[:, kb, :])
                else:
                    nc.sync.dma_start(xw[:], ap)
                return xw

            # ---- background thunk machinery (EDF) ----
            bg_wait = []          # appended in nondecreasing avail order

            bg_heap = []          # (deadline, seq, thunk)
            bg_seq = [0]

            def bg_schedule(avail, deadline, thunks):
                for t in thunks:
                    bg_wait.append((avail, deadline, bg_seq[0], t))
                    bg_seq[0] += 1

            def bg_tick(idx, next_idx, pace=PACE):
                while bg_wait and bg_wait[0][0] <= idx:
                    _, dl, seq, t = bg_wait.pop(0)
                    heapq.heappush(bg_heap, (dl, seq, t))
                while bg_heap and bg_heap[0][0] <= next_idx:
                    heapq.heappop(bg_heap)[2]()
                for _ in range(pace):
                    if bg_heap:
                        heapq.heappop(bg_heap)[2]()

            def bg_drain_all(idx):
                while bg_wait and bg_wait[0][0] <= idx:
                    _, dl, seq, t = bg_wait.pop(0)
                    heapq.heappush(bg_heap, (dl, seq, t))
                while bg_heap:
                    heapq.heappop(bg_heap)[2]()

            # ---- qkv production thunks ----
            def kT_db_thunks(xw_box, w, db):
                """k projection block (window w, pair db): 4 mm + 1 copy."""
                box = [None]
                thunks = []

                def mm_t(kb0):
                    if kb0 == 0:
                        box[0] = ps_bg.tile([P, 512], f32, tag="bg",
                                            name=f"pk{w}_{db}")
                    for kb in (kb0, kb0 + 1):
                        nc.tensor.matmul(
                            box[0][:],
                            lhsT=wk_box[0][:, kb, db * P:(db + 1) * P],
                            rhs=xw_box[0][:, kb, :],
                            start=(kb == 0), stop=(kb == KB - 1))
                for kb0 in range(0, KB, 2):
                    thunks.append(lambda kb0=kb0: mm_t(kb0))

                def cp_t():
                    nc.vector.tensor_copy(
                        out=kT_sb[:, db, w * 512:(w + 1) * 512],
                        in_=box[0][:])
                thunks.append(cp_t)
                return thunks

            def v_m_thunks(xw_box, w, mc):
                """v for m-tile 4w+mc: 4 mm + 1 copy."""
                m = w * 4 + mc
                box = [None]
                thunks = []

                def mm_t(kb0):
                    if kb0 == 0:
                        box[0] = ps_bg.tile([P, 512], f32, tag="bg",
                                            name=f"pv{m}")
                    for kb in (kb0, kb0 + 1):
                        nc.tensor.matmul(
                            box[0][:],
                            lhsT=xw_box[0][:, kb, mc * P:(mc + 1) * P],
                            rhs=wv_box[0][:, kb, :],
                            start=(kb == 0), stop=(kb == KB - 1))
                for kb0 in range(0, KB, 2):
                    thunks.append(lambda kb0=kb0: mm_t(kb0))

                def cp_t():
                    nc.vector.tensor_copy(
                        out=v_sb[:, m, :, 0:HD],
                        in_=box[0][:].rearrange("p (h d) -> p h d", h=NHC))
                thunks.append(cp_t)
                return thunks

            qT_tiles = [None] * NCH

            def q_db_thunks(jn, xw_box, qt, db):
                """q block db of chunk jn: 4 mm + 1 copy (no zero pad)."""
                box = [None]
                thunks = []

                def mm_t(kb0):
                    if kb0 == 0:
                        box[0] = ps_bg.tile([P, 512], f32, tag="bg",
                                            name=f"pq{jn}_{db}")
                    for kb in (kb0, kb0 + 1):
                        nc.tensor.matmul(
                            box[0][:],
                            lhsT=wq_sb[:, kb, db * P:(db + 1) * P],
                            rhs=xw_box[0][:, kb, :],
                            start=(kb == 0), stop=(kb == KB - 1))
                for kb0 in range(0, KB, 2):
                    thunks.append(lambda kb0=kb0: mm_t(kb0))

                def cp_t():
                    nc.vector.tensor_copy(out=qt[:, db, :], in_=box[0][:])
                thunks.append(cp_t)
                return thunks

            def schedule_qT(jn, avail, deadline):
                qt = qT_pool.tile([P, DB, 512], bf16, tag="qTc",
                                  name=f"qT{jn}")
                qT_tiles[jn] = qt
                thunks = []
                for db in range(DB):
                    thunks.extend(q_db_thunks(jn, xw_boxes[jn], qt, db))
                bg_schedule(avail, deadline, thunks)

            def emit_proj_thunks(j):
                """Projection of chunk j (at_j is bf16)."""
                at_j = at_tiles[j]
                thunks = []
                box = [None]
                for ns in range(4):
                    for ec in range(2):
                        def mm_t(ns, ec, cb0):
                            if cb0 == 0:
                                box[0] = ps_bg.tile([P, 512], f32, tag="bg",
                                                    name=f"pp{ns}_{ec}")
                            for cb in (cb0, cb0 + 1):
                                nc.tensor.matmul(
                                    box[0][:],
                                    lhsT=at_j[:, cb, ns * P:(ns + 1) * P],
                                    rhs=wp_box[0][:, cb,
                                                  ec * 512:(ec + 1) * 512],
                                    start=(cb == 0), stop=(cb == DB - 1))
                        for cb0 in range(0, DB, 2):
                            thunks.append(
                                lambda ns=ns, ec=ec, cb0=cb0: mm_t(ns, ec, cb0))
                        def cp_t(ns=ns, ec=ec):
                            osb = out_pool.tile([P, 512], f32, tag="osb",
                                                name=f"osb{ns}_{ec}")
                            nc.vector.tensor_copy(out=osb[:], in_=box[0][:])
                            nc.sync.dma_start(
                                out.ap()[j * 512 + ns * P:
                                         j * 512 + (ns + 1) * P,
                                         ec * 512:(ec + 1) * 512],
                                osb[:])
                        thunks.append(cp_t)
                return thunks

            wk_box = [None]
            wv_box = [None]
            wp_box = [None]
            xw_boxes = [[None] for _ in range(NCH)]

            # ---- prelude: DMAs + minimal serial work for gstep 0 ----
            wkv_scope = tc.tile_pool(name="wkv", bufs=1)
            wkv_pool = wkv_scope.__enter__()
            wk_sb = wkv_pool.tile([P, KB, DC], bf16, tag="wk")
            wv_sb = wkv_pool.tile([P, KB, DC], bf16, tag="wv")
            wk_box[0] = wk_sb
            wv_box[0] = wv_sb
            xw0 = xw_pool.tile([P, KB, 512], bf16, tag="xw", name="xw_kv0")
            xw0_ap = xh.ap()[:, 0, :, :]
            # first k-blocks land first so kT(w0,db0)/q0(db0) start early
            for kb in range(KB):
                nc.sync.dma_start(wk_sb[:, kb, :], wkh.ap()[:, kb, :])
                nc.sync.dma_start(xw0[:, kb, :], xw0_ap[:, kb, :])
                nc.sync.dma_start(wq_sb[:, kb, :], wqh.ap()[:, kb, :])
            nc.sync.dma_start(wv_sb[:], wvh.ap())
            xw_boxes[0][0] = xw0
            # prefetch x windows 1-3; kv and qT production share these
            # tiles (x is DMA'd once per window)
            for w in range(1, NCH):
                xw_boxes[w][0] = load_xw(w, f"kv{w}")

            # serial: kT(w0, db0) and q0(db0) unblock gstep 0
            for t in kT_db_thunks(xw_boxes[0], 0, 0):
                t()
            qt0 = qT_pool.tile([P, DB, 512], bf16, tag="qTc", name="qT0")
            qT_tiles[0] = qt0
            for t in q_db_thunks(0, xw_boxes[0], qt0, 0):
                t()
            # everything else is deadline-scheduled: kT(w,db) first needed
            # by S(p=db, g=4w) at gstep 16db+4w; v(m) by av at gstep m+3;
            # q0(db) by S(p=db) at 16db
            for w in range(NCH):
                for db in range(DB):
                    if (w, db) == (0, 0):
                        continue
                    bg_schedule(0, 16 * db + 4 * w,
                                kT_db_thunks(xw_boxes[w], w, db))
                for mc in range(4):
                    bg_schedule(0, 4 * w + mc + 2,
                                v_m_thunks(xw_boxes[w], w, mc))
            for db in range(1, DB):
                bg_schedule(0, 16 * db, q_db_thunks(0, xw_boxes[0], qt0, db))

            expS_scope = tc.tile_pool(name="expS", bufs=5)
            expS_pool = expS_scope.__enter__()
            at_scope = tc.tile_pool(name="at", bufs=2)
            at_pool = at_scope.__enter__()
            small_scope = tc.tile_pool(name="small", bufs=1)
            small_pool = small_scope.__enter__()
            out_scope = tc.tile_pool(name="outsb", bufs=2)
            out_pool = out_scope.__enter__()

            # ---- flat gstep stream: (chunk j, head-pair p, m-tile g) ----
            gsteps = [(j, p, g) for j in range(NCH) for p in range(DB)
                      for g in range(MT)]
            CHUNK = DB * MT
            at_tiles = [None] * NCH
            eS_q = {}
            po_pairs = {}
            pending_p1 = []
            pending_rcp = []
            pending_ep = []

            def emit_S(j, p, g):
                S = ps_S.tile([P, 1024], f32, tag="S", name=f"S{p}_{g}")
                qt = qT_tiles[j]
                nc.tensor.matmul(S[:, 0:512],
                                 lhsT=kT_sb[0:HD, p, g * P:(g + 1) * P],
                                 rhs=qt[0:HD, p, :],
                                 start=True, stop=True)
                nc.tensor.matmul(S[:, 512:1024],
                                 lhsT=kT_sb[HD:P, p, g * P:(g + 1) * P],
                                 rhs=qt[HD:P, p, :],
                                 start=True, stop=True)
                return S

            def emit_av(idx2):
                j, p, g = gsteps[idx2]
                eSp = eS_q.pop((j, p, g))
                key = (j, p)
                if key not in po_pairs:
                    po_pairs[key] = [
                        ps_o.tile([P, 512], f32, tag="o",
                                  name=f"po{j}_{p}_{h}")
                        for h in range(2)]
                po0, po1 = po_pairs[key]
                nc.tensor.matmul(po0[0:HD + 1, :],
                                 lhsT=v_sb[:, g, 2 * p, :],
                                 rhs=eSp[:, 0:512],
                                 start=(g == 0), stop=(g == MT - 1))
                nc.tensor.matmul(po1[0:HD + 1, :],
                                 lhsT=v_sb[:, g, 2 * p + 1, :],
                                 rhs=eSp[:, 512:1024],
                                 start=(g == 0), stop=(g == MT - 1))
                if g == MT - 1:
                    pending_p1.append([3, po0, po1, j, p])

            def emit_epilogue_p1(po0, po1, j, p):
                """Free the po banks; stage denominator rows via DMA."""
                oT0 = small_pool.tile([HD + 1, 512], f32, tag="oT0",
                                      name="oT0")
                oT1 = small_pool.tile([HD + 1, 512], f32, tag="oT1",
                                      name="oT1")
                nc.vector.tensor_copy(out=oT0[:], in_=po0[0:HD + 1, :])
                nc.vector.tensor_copy(out=oT1[:], in_=po1[0:HD + 1, :])
                nc.sync.dma_start(den_sb[0:1, :], oT0[HD:HD + 1, :])
                nc.sync.dma_start(den_sb[32:33, :], oT1[HD:HD + 1, :])
                if debug and (j, p) == (0, 0):
                    nc.sync.dma_start(dbg_po.ap()[:, 0:512], oT0[:])
                    nc.sync.dma_start(dbg_po.ap()[:, 512:1024], oT1[:])
                return oT0, oT1

            def emit_epilogue_recip():
                rcp = small_pool.tile([33, 512], f32r, tag="rcp", name="rcp")
                with nc.allow_low_precision(reason="softmax recip to f32r"):
                    nc.vector.reciprocal(rcp[:], den_sb[:])
                return rcp

            def emit_epilogue_p2(j, p, oT0, oT1, rcp):
                at = at_tiles[j]
                with nc.allow_low_precision(reason="softmax normalize bf16"):
                    bcp0 = ps_bg.tile([P, 512], f32, tag="bg", name="bcp0")
                    nc.tensor.matmul(bcp0[0:HD, :],
                                     lhsT=ones_r[0:1, :],
                                     rhs=rcp[0:1, :],
                                     start=True, stop=True)
                    nc.vector.tensor_tensor(
                        out=at[0:HD, p, :], in0=oT0[0:HD, :],
                        in1=bcp0[0:HD, :], op=MULT)
                    bcp1 = ps_bg.tile([P, 512], f32, tag="bg", name="bcp1")
                    nc.tensor.matmul(bcp1[0:HD, :],
                                     lhsT=ones_r[32:33, :],
                                     rhs=rcp[32:33, :],
                                     start=True, stop=True)
                    tmp1 = small_pool.tile([HD, 512], bf16, tag="tmp1",
                                           name="tmp1")
                    nc.vector.tensor_tensor(
                        out=tmp1[:], in0=oT1[0:HD, :],
                        in1=bcp1[0:HD, :], op=MULT)
                    nc.sync.dma_start(at[HD:P, p, :], tmp1[:])

            def drain_queues():
                # later stages first: den_sb is shared, so pair n's recip
                # must be emitted before pair n+1's den DMAs
                for ep in pending_ep:
                    ep[0] -= 1
                while pending_ep and pending_ep[0][0] <= 0:
                    _, j, p, oT0, oT1, rcp = pending_ep.pop(0)
                    emit_epilogue_p2(j, p, oT0, oT1, rcp)
                for ep in pending_rcp:
                    ep[0] -= 1
                while pending_rcp and pending_rcp[0][0] <= 0:
                    _, j, p, oT0, oT1 = pending_rcp.pop(0)
                    rcp = emit_epilogue_recip()
                    # bcp matmuls enter the in-order PE queue only after
                    # the 3.3us batched DVE reciprocal has surely finished
                    pending_ep.append([9, j, p, oT0, oT1, rcp])
                for ep in pending_p1:
                    ep[0] -= 1
                while pending_p1 and pending_p1[0][0] <= 0:
                    _, po0, po1, j, p = pending_p1.pop(0)
                    oT0, oT1 = emit_epilogue_p1(po0, po1, j, p)
                    pending_rcp.append([2, j, p, oT0, oT1])

            def start_chunk(c, idx):
                if c == 0:
                    wp_scope = tc.tile_pool(name="wp", bufs=1)
                    wp_pool = wp_scope.__enter__()
                    wp_box.append(wp_scope)  # keep scope alive
                    wp_sb = wp_pool.tile([P, DB, D], bf16, tag="wp")
                    nc.sync.dma_start(wp_sb[:], wph.ap())
                    wp_box[0] = wp_sb
                if debug and c == 1:
                    nc.sync.dma_start(dbg_kT.ap(), kT_sb[:])
                    nc.sync.dma_start(dbg_v.ap(), v_sb[:])
                    nc.sync.dma_start(dbg_qT.ap(), qT_tiles[0][:])
                if debug and c == 2:
                    nc.sync.dma_start(dbg_at.ap(), at_tiles[0][:])
                at_tiles[c] = at_pool.tile([P, DB, 512], bf16, tag="at",
                                           name=f"at{c}")
                if c + 1 < NCH:
                    schedule_qT(c + 1, idx, idx + CHUNK)
                if c >= 1:
                    bg_schedule(idx + 16, idx + CHUNK,
                                emit_proj_thunks(c - 1))

            start_chunk(0, 0)
            S_next = emit_S(*gsteps[0])
            for idx, (j, p, g) in enumerate(gsteps):
                S_cur = S_next
                eS = expS_pool.tile([P, 1024], bf16, tag="e",
                                    name=f"eS{p}_{g}")
                nc.scalar.activation(eS[:], S_cur[:], Exp, scale=SCALE)
                if debug and idx == 0:
                    nc.sync.dma_start(dbg_eS.ap(), eS[:])
                eS_q[(j, p, g)] = eS
                if idx >= AV_LAG:
                    emit_av(idx - AV_LAG)
                drain_queues()
                if idx + 1 < len(gsteps):
                    if (idx + 1) % CHUNK == 0:
                        bg_drain_all(idx)
                        start_chunk((idx + 1) // CHUNK, idx + 1)
                    bg_tick(idx, idx + 1)
                    S_next = emit_S(*gsteps[idx + 1])

            for idx2 in range(len(gsteps) - AV_LAG, len(gsteps)):
                emit_av(idx2)
            while pending_p1 or pending_rcp or pending_ep:
                drain_queues()
            bg_drain_all(10 ** 9)

            # final chunk's projection
            for t in emit_proj_thunks(NCH - 1):
                t()

            if len(wp_box) > 1:
                wp_box[1].__exit__(None, None, None)
            out_scope.__exit__(None, None, None)
            small_scope.__exit__(None, None, None)
            at_scope.__exit__(None, None, None)
            expS_scope.__exit__(None, None, None)
            wkv_scope.__exit__(None, None, None)

    nc.compile()
    return nc


_CACHE: dict = {}


def _get_program():
    if "nc" not in _CACHE:
        _CACHE["nc"] = build_program()
    return _CACHE["nc"]


def make_in_maps(x, w_qkv, w_proj):
    """Host-side sharding: per-core input dict (bf16)."""
    bf = ml_dtypes.bfloat16
    x = np.asarray(x, dtype=np.float32)
    w_qkv = np.asarray(w_qkv, dtype=np.float32)
    w_proj = np.asarray(w_proj, dtype=np.float32)
    in_maps = []
    for core in range(8):
        b, g = divmod(core, 2)
        gsl = slice(g * DC, (g + 1) * DC)
        xT = x[b].T.astype(bf)                                    # [D, N]
        wqT = w_qkv[0 * D:1 * D][gsl].T.astype(bf)                # [D, DC]
        wkT = w_qkv[1 * D:2 * D][gsl].T.astype(bf)
        wvT = w_qkv[2 * D:3 * D][gsl].T.astype(bf)
        wpT = w_proj[:, gsl].T.astype(bf)                         # [DC, D]
        in_maps.append({
            # [P, NCH, KB, 512]: contiguous 8KB lines per window
            "xh": np.ascontiguousarray(
                xT.reshape(KB, P, NCH, 512).transpose(1, 2, 0, 3)),
            "wqh": np.ascontiguousarray(
                wqT.reshape(KB, P, DC).transpose(1, 0, 2)),
            "wkh": np.ascontiguousarray(
                wkT.reshape(KB, P, DC).transpose(1, 0, 2)),
            "wvh": np.ascontiguousarray(
                wvT.reshape(KB, P, DC).transpose(1, 0, 2)),
            "wph": np.ascontiguousarray(
                wpT.reshape(DB, P, D).transpose(1, 0, 2)),
        })
    return in_maps


def run(x, w_qkv, w_proj, b_proj, **spmd_kwargs):
    nc = _get_program()
    in_maps = make_in_maps(x, w_qkv, w_proj)
    res = run_bass_kernel_spmd(nc, in_maps, list(range(8)), **spmd_kwargs)
    b_proj = np.asarray(b_proj, dtype=np.float32)
    outp = np.empty((B, N, D), dtype=np.float32)
    for b in range(B):
        outp[b] = (res.results[2 * b]["out"] + res.results[2 * b + 1]["out"]
                   + b_proj[None, :])
    return outp, res


def kernel(x, w_qkv, w_proj, b_proj):
    outp, _ = run(x, w_qkv, w_proj, b_proj)
    return outp
